# revision 42
# baseline (speedup 1.0000x reference)
"""Trainium2 Bass kernel for nn_AttentionModel (Kool-style TSP attention model).

Data-parallel over 8 NeuronCores: each core processes B/8 = 64 samples,
fp32 throughout, transposed activation layouts (features on partitions).
The TSP tour cost is reconstructed on the host from pi (pure indexing).
"""

import numpy as np

import concourse.bass as bass
import concourse.mybir as mybir
import concourse.tile as tile
from concourse import bacc
from concourse.bass_utils import run_bass_kernel_spmd
from concourse.masks import make_identity

F32 = mybir.dt.float32
I32 = mybir.dt.int32
U32 = mybir.dt.uint32
I16 = mybir.dt.int16
AF = mybir.ActivationFunctionType
OP = mybir.AluOpType

B, N, D, H, L, FF = 512, 100, 128, 8, 2, 512
DK = D // H          # 16
NCORE = 8
BC = B // NCORE      # 64 samples per core
BN = BC * N          # 6400
NPAD = 128
CLIP = 10.0
NEG = -1e9
NF = 1.0 / float(np.sqrt(D))      # decoder norm factor
SC = 1.0 / float(np.sqrt(DK))     # encoder attention scale
EPS = 1e-5

CH = [(o, min(512, BN - o)) for o in range(0, BN, 512)]


def build_nc(debug=False, use_cc=True):
    nc = bacc.Bacc("TRN2", target_bir_lowering=False, debug=False,
                   num_devices=NCORE if use_cc else 1)

    dx = nc.dram_tensor("x", [BC, N, 2], F32, kind="ExternalInput")
    dinit_W = nc.dram_tensor("init_W", [2, D], F32, kind="ExternalInput")
    dinit_b = nc.dram_tensor("init_b", [D], F32, kind="ExternalInput")
    dqkv = nc.dram_tensor("enc_qkv_W", [L, D, 3 * D], F32, kind="ExternalInput")
    doutW = nc.dram_tensor("enc_out_W", [L, D, D], F32, kind="ExternalInput")
    dbn1 = nc.dram_tensor("enc_bn1", [L, 2, D], F32, kind="ExternalInput")
    dbn2 = nc.dram_tensor("enc_bn2", [L, 2, D], F32, kind="ExternalInput")
    dff1W = nc.dram_tensor("enc_ff1_W", [L, D, FF], F32, kind="ExternalInput")
    dff1b = nc.dram_tensor("enc_ff1_b", [L, FF], F32, kind="ExternalInput")
    dff2W = nc.dram_tensor("enc_ff2_W", [L, FF, D], F32, kind="ExternalInput")
    dff2b = nc.dram_tensor("enc_ff2_b", [L, D], F32, kind="ExternalInput")
    dWph = nc.dram_tensor("W_placeholder", [2 * D], F32, kind="ExternalInput")
    dWnod = nc.dram_tensor("proj_nodes_W", [D, 3 * D], F32, kind="ExternalInput")
    dWfix = nc.dram_tensor("proj_fixed_W", [D, D], F32, kind="ExternalInput")
    dWstep = nc.dram_tensor("proj_step_W", [2 * D, D], F32, kind="ExternalInput")
    dWout = nc.dram_tensor("proj_out_W", [D, D], F32, kind="ExternalInput")
    dgidx = nc.dram_tensor("gidx", [128, BC // 16], I16, kind="ExternalInput")
    de4 = nc.dram_tensor("e4c", [4, 128], F32, kind="ExternalInput")
    dhm = nc.dram_tensor("hmask", [128, 8], F32, kind="ExternalInput")
    de16 = nc.dram_tensor("e16c", [16, 128], F32, kind="ExternalInput")

    dll = nc.dram_tensor("ll", [BC], F32, kind="ExternalOutput")
    dpi = nc.dram_tensor("pi", [BC, N], I32, kind="ExternalOutput")
    if debug:
        ddbg = nc.dram_tensor("dbg_h", [D, BN], F32, kind="ExternalOutput")
        ddbg_l = nc.dram_tensor("dbg_l", [BC, N], F32, kind="ExternalOutput")
        ddbg_g = nc.dram_tensor("dbg_g", [D, BC], F32, kind="ExternalOutput")
        ddbg_a = nc.dram_tensor("dbg_a", [128, 16 * NPAD], F32,
                                kind="ExternalOutput")
    dpc2 = nc.dram_tensor("pc2", [BC * 101, 832], F32)
    dpc1 = nc.dram_tensor("pc1", [BC * N, 832], F32)
    cc_bufs = []
    if use_cc:
        for i in range(2 * L):
            cc_bufs.append((nc.dram_tensor(f"ccin{i}", [D, 2], F32),
                            nc.dram_tensor(f"ccout{i}", [D, 2], F32)))

    with tile.TileContext(nc) as tc:
        with tc.tile_pool(name="P", bufs=1) as P, \
             tc.tile_pool(name="SCR", bufs=1) as SCR, \
             tc.tile_pool(name="SM", bufs=2) as SM, \
             tc.tile_pool(name="PS", bufs=3, space="PSUM") as PS:
            _build_model(nc, tc, P, SCR, SM, PS, locals(), debug, use_cc,
                         cc_bufs)
    nc.compile()
    return nc


def _build_model(nc, tc, P, SCR, SM, PS, dd, debug, use_cc, cc_bufs):
    dx, dinit_W, dinit_b = dd["dx"], dd["dinit_W"], dd["dinit_b"]
    dqkv, doutW, dbn1, dbn2 = dd["dqkv"], dd["doutW"], dd["dbn1"], dd["dbn2"]
    dff1W, dff1b, dff2W, dff2b = dd["dff1W"], dd["dff1b"], dd["dff2W"], dd["dff2b"]
    dWph, dWnod, dWfix, dWstep, dWout = (dd["dWph"], dd["dWnod"], dd["dWfix"],
                                         dd["dWstep"], dd["dWout"])
    dgidx, dll, dpi = dd["dgidx"], dd["dll"], dd["dpi"]

    # ---------------- weights ----------------
    ident = P.tile([128, 128], F32, tag="ident")
    make_identity(nc, ident)

    w_init = P.tile([2, D], F32, tag="w_init")
    nc.sync.dma_start(out=w_init, in_=dinit_W.ap())
    b_init = P.tile([D, 1], F32, tag="b_init")
    nc.sync.dma_start(out=b_init, in_=dinit_b.ap().rearrange("(d o) -> d o", o=1))
    w_qkv = P.tile([D, L, 3 * D], F32, tag="w_qkv")
    nc.sync.dma_start(out=w_qkv[:], in_=dqkv.ap().transpose([1, 0, 2]))
    w_out = P.tile([D, L, D], F32, tag="w_out")
    nc.sync.dma_start(out=w_out[:], in_=doutW.ap().transpose([1, 0, 2]))
    w_ff1 = P.tile([D, L, FF], F32, tag="w_ff1")
    nc.sync.dma_start(out=w_ff1[:], in_=dff1W.ap().transpose([1, 0, 2]))
    b_ff1 = P.tile([128, L, FF // 128], F32, tag="b_ff1")
    nc.sync.dma_start(
        out=b_ff1[:],
        in_=dff1b.ap().rearrange("l (c p) -> p l c", p=128))
    w_ff2 = P.tile([128, L, FF // 128, D], F32, tag="w_ff2")
    nc.sync.dma_start(
        out=w_ff2[:],
        in_=dff2W.ap().rearrange("l (c p) d -> p l c d", p=128))
    b_ff2 = P.tile([D, L], F32, tag="b_ff2")
    nc.sync.dma_start(out=b_ff2[:], in_=dff2b.ap().transpose([1, 0]))
    bn_gb1 = P.tile([D, L, 2], F32, tag="bn_gb1")  # [d, l, gamma/beta]
    nc.sync.dma_start(out=bn_gb1[:], in_=dbn1.ap().transpose([2, 0, 1]))
    bn_gb2 = P.tile([D, L, 2], F32, tag="bn_gb2")
    nc.sync.dma_start(out=bn_gb2[:], in_=dbn2.ap().transpose([2, 0, 1]))
    w_nod = P.tile([D, 3 * D], F32, tag="w_nod")
    nc.sync.dma_start(out=w_nod[:], in_=dWnod.ap())
    w_fix = P.tile([D, D], F32, tag="w_fix")
    nc.sync.dma_start(out=w_fix[:], in_=dWfix.ap())
    w_step = P.tile([128, 2, D], F32, tag="w_step")
    nc.sync.dma_start(out=w_step[:],
                      in_=dWstep.ap().rearrange("(c p) d -> p c d", p=128))
    w_outd = P.tile([D, D], F32, tag="w_outd")
    nc.sync.dma_start(out=w_outd[:], in_=dWout.ap())
    wph = P.tile([128, 2], F32, tag="wph")
    nc.sync.dma_start(out=wph[:], in_=dWph.ap().rearrange("(c p) -> p c", p=128))
    gidx_sb = P.tile([128, BC // 16], I16, tag="gidx")
    nc.sync.dma_start(out=gidx_sb[:], in_=dgidx.ap())

    # mask injector: e4[s, 32s+h] = NEG for h<8 (host-provided)
    e4 = P.tile([4, 128], F32, tag="e4")
    nc.sync.dma_start(out=e4[:], in_=dd["de4"].ap())
    hmask = P.tile([128, 8], F32, tag="hmask")
    nc.sync.dma_start(out=hmask[:], in_=dd["dhm"].ap())
    e16 = P.tile([16, 128], F32, tag="e16")
    nc.sync.dma_start(out=e16[:], in_=dd["de16"].ap())
    iota_n = P.tile([64, N], I32, tag="iota_n")
    nc.gpsimd.iota(iota_n, pattern=[[1, N]], base=0, channel_multiplier=0)
    iota_f = P.tile([64, N], F32, tag="iota_f")
    nc.vector.tensor_copy(out=iota_f[:], in_=iota_n[:])
    # row of b*100 offsets [1, 64]
    boff_i = P.tile([1, BC], I32, tag="boff_i")
    nc.gpsimd.iota(boff_i, pattern=[[N, BC]], base=0, channel_multiplier=0)
    boff_f = P.tile([1, BC], F32, tag="boff_f")
    nc.vector.tensor_copy(out=boff_f[:], in_=boff_i[:])
    boffc_i = P.tile([BC, 1], I32, tag="boffc_i")
    nc.gpsimd.iota(boffc_i, pattern=[[1, 1]], base=0, channel_multiplier=N)
    boffc_f = P.tile([BC, 1], F32, tag="boffc_f")
    nc.vector.tensor_copy(out=boffc_f[:], in_=boffc_i[:])

    eps_t = P.tile([D, 1], F32, tag="eps_t")
    nc.vector.memset(eps_t, EPS)

    hT = P.tile([D, BN], F32, tag="hT")

    # ---------------- h0 = (x @ init_W + init_b)^T ----------------
    for (o, w) in CH:
        xTs = SCR.tile([2, 512], F32, tag="xm")
        nc.sync.dma_start(
            out=xTs[:, :w],
            in_=bass.AP(tensor=dx.ap().tensor, offset=2 * o,
                        ap=[[1, 2], [2, w]]))
        ps = PS.tile([128, 512], F32, tag="A")
        nc.tensor.matmul(ps[:, :w], lhsT=w_init[:], rhs=xTs[:, :w],
                         start=True, stop=True)
        nc.scalar.activation(out=hT[:, o:o + w], in_=ps[:, :w],
                             func=AF.Identity, bias=b_init[:], scale=1.0)

    # ---------------- BatchNorm helper (in place on [D, BN]) ----------------
    def bn_pass(t, l, which):
        nsub = BN // 128  # 50 subgroups of 128 for bn_stats
        stats = SM.tile([D, nsub, 6], F32, tag="bnstats")
        tv = t[:].rearrange("d (s c) -> d s c", c=128)
        for s in range(nsub):
            nc.vector.bn_stats(out=stats[:, s, :], in_=tv[:, s, :])
        mv = SM.tile([D, 2], F32, tag="bnmv")
        nc.vector.bn_aggr(out=mv[:], in_=stats[:])
        if use_cc:
            # cross-core stats: allreduce (mean, var + mean^2), divide by 8
            pay = SM.tile([D, 2], F32, tag="ccpay")
            nc.vector.tensor_copy(out=pay[:, 0:1], in_=mv[:, 0:1])
            nc.vector.scalar_tensor_tensor(
                out=pay[:, 1:2], in0=mv[:, 0:1], scalar=mv[:, 0:1],
                in1=mv[:, 1:2], op0=OP.mult, op1=OP.add)
            cin, cout = cc_bufs[2 * l + which]
            nc.sync.dma_start(out=cin.ap(), in_=pay[:])
            nc.gpsimd.collective_compute(
                "AllReduce", OP.add, replica_groups=[list(range(NCORE))],
                ins=[cin.ap()], outs=[cout.ap()])
            nc.sync.dma_start(out=pay[:], in_=cout.ap())
            # mean = pay0/8 ; var = pay1/8 - mean^2
            nc.scalar.mul(out=mv[:, 0:1], in_=pay[:, 0:1], mul=1.0 / NCORE)
            msq = SM.tile([D, 1], F32, tag="ccmsq")
            nc.vector.tensor_mul(out=msq[:], in0=mv[:, 0:1], in1=mv[:, 0:1])
            nc.vector.tensor_scalar(
                out=mv[:, 1:2], in0=pay[:, 1:2], scalar1=1.0 / NCORE,
                scalar2=msq[:], op0=OP.mult, op1=OP.subtract)
        rstd = SM.tile([D, 1], F32, tag="bnrstd")
        nc.scalar.activation(out=rstd[:], in_=mv[:, 1:2], func=AF.Sqrt,
                             bias=eps_t[:], scale=1.0)
        nc.vector.reciprocal(out=rstd[:], in_=rstd[:])
        scale = SM.tile([D, 1], F32, tag="bnscale")
        nc.vector.tensor_mul(out=scale[:], in0=rstd[:],
                             in1=(bn_gb1 if which == 0 else bn_gb2)[:, l, 0:1])
        shift = SM.tile([D, 1], F32, tag="bnshift")
        nc.vector.tensor_mul(out=shift[:], in0=mv[:, 0:1], in1=scale[:])
        nc.vector.tensor_scalar(
            out=shift[:], in0=(bn_gb1 if which == 0 else bn_gb2)[:, l, 1:2], scalar1=shift[:],
            scalar2=None, op0=OP.subtract)
        for i, (o, w) in enumerate(CH):
            if i % 2 == 0:
                nc.scalar.activation(out=t[:, o:o + w], in_=t[:, o:o + w],
                                     func=AF.Identity, bias=shift[:],
                                     scale=scale[:])
            else:
                nc.vector.tensor_scalar(
                    out=t[:, o:o + w], in0=t[:, o:o + w], scalar1=scale[:],
                    scalar2=shift[:], op0=OP.mult, op1=OP.add)

    # ---------------- encoder ----------------
    for l in range(L):
        qT = SCR.tile([D, BN], F32, tag="qT")
        kT = SCR.tile([D, BN], F32, tag="kT")
        for blk, dst in ((0, qT), (1, kT)):
            for i, (o, w) in enumerate(CH):
                ps = PS.tile([128, 512], F32, tag="A")
                nc.tensor.matmul(ps[:, :w],
                                 lhsT=w_qkv[:, l, blk * 128:(blk + 1) * 128],
                                 rhs=hT[:, o:o + w], start=True, stop=True)
                if i % 2 == 0:
                    nc.scalar.copy(out=dst[:, o:o + w], in_=ps[:, :w])
                else:
                    nc.vector.tensor_copy(out=dst[:, o:o + w], in_=ps[:, :w])
        v_nat = SCR.tile([N, BC * D], F32, tag="v_nat")
        for b in range(BC):
            ps = PS.tile([N, 128], F32, tag="A")
            nc.tensor.matmul(ps[:], lhsT=hT[:, b * N:(b + 1) * N],
                             rhs=w_qkv[:, l, 256:384], start=True, stop=True)
            if b % 2 == 0:
                nc.scalar.copy(out=v_nat[:, b * D:(b + 1) * D], in_=ps[:])
            else:
                nc.vector.tensor_copy(out=v_nat[:, b * D:(b + 1) * D], in_=ps[:])

        # attention in groups of 4 samples (zero-embedded khat)
        GS = 2
        khat = SCR.tile([128, GS * 800], F32, tag="khat")
        nc.vector.memset(khat, 0.0)
        for bg in range(BC // GS):
            b0 = bg * GS
            for h in range(H):
                nc.vector.tensor_scalar_mul(
                    out=khat[:, :].rearrange(
                        "p (bb m) -> p bb m", bb=GS)[:, :, 100 * h:100 * h + 100],
                    in0=kT[:, b0 * N:(b0 + GS) * N].rearrange(
                        "p (bb m) -> p bb m", bb=GS),
                    scalar1=hmask[:, h:h + 1])
            att_s = SCR.tile([N, GS * 800], F32, tag="att_s")
            for bb in range(GS):
                b = b0 + bb
                ps = PS.tile([N, 800], F32, tag="A")
                for (o, w) in ((0, 512), (512, 288)):
                    nc.tensor.matmul(
                        ps[:, o:o + w], lhsT=qT[:, b * N:(b + 1) * N],
                        rhs=khat[:, bb * 800 + o: bb * 800 + o + w],
                        start=True, stop=True, skip_group_check=True)
                nc.scalar.activation(out=att_s[:, bb * 800:(bb + 1) * 800],
                                     in_=ps[:], func=AF.Exp, scale=SC)
            ssum = SM.tile([N, GS * 8], F32, tag="ssum")
            nc.vector.tensor_reduce(
                out=ssum[:],
                in_=att_s[:].rearrange("p (bb h m) -> p (bb h) m", bb=GS, h=H),
                axis=mybir.AxisListType.X, op=OP.add)
            nc.vector.reciprocal(out=ssum[:], in_=ssum[:])
            nc.vector.scalar_tensor_tensor(
                out=att_s[:].rearrange("p (bb h m) -> p (bb h) m", bb=GS, h=H),
                in0=att_s[:].rearrange("p (bb h m) -> p (bb h) m", bb=GS, h=H),
                scalar=1.0,
                in1=ssum[:].unsqueeze(2).broadcast_to([N, GS * 8, 100]),
                op0=OP.mult, op1=OP.mult)
            # transpose att blocks (batched per 4 heads); o = att @ v
            o_all = SCR.tile([N, GS * D], F32, tag="o_all")
            for bb in range(GS):
                b = b0 + bb
                atT = SCR.tile([100, 800], F32, tag="xm")
                for hg in range(2):
                    pst = PS.tile([128, 400], F32, tag="A")
                    for hh in range(4):
                        h = 4 * hg + hh
                        nc.tensor.transpose(
                            pst[:100, hh * 100:(hh + 1) * 100],
                            att_s[:, bb * 800 + h * 100: bb * 800 + (h + 1) * 100],
                            ident[:100, :100])
                    if hg == 0:
                        nc.scalar.copy(out=atT[:, :400], in_=pst[:100, :])
                    else:
                        nc.vector.tensor_copy(out=atT[:, 400:], in_=pst[:100, :])
                pso = PS.tile([N, D], F32, tag="A")
                for h in range(H):
                    nc.tensor.matmul(
                        pso[:, h * DK:(h + 1) * DK],
                        lhsT=atT[:, h * 100:(h + 1) * 100],
                        rhs=v_nat[:, b * D + h * DK: b * D + (h + 1) * DK],
                        start=True, stop=True, skip_group_check=True)
                nc.vector.tensor_copy(out=o_all[:, bb * D:(bb + 1) * D],
                                      in_=pso[:])
            # transpose o per sample and project + residual into hT
            for bb in range(GS):
                b = b0 + bb
                pst = PS.tile([128, 100], F32, tag="A")
                nc.tensor.transpose(pst[:], o_all[:, bb * D:(bb + 1) * D],
                                    ident[:100, :100])
                oTs = SM.tile([128, 100], F32, tag="oTs")
                nc.vector.tensor_copy(out=oTs[:], in_=pst[:])
                ps2 = PS.tile([128, 100], F32, tag="A")
                nc.tensor.matmul(ps2[:], lhsT=w_out[:, l, :], rhs=oTs[:],
                                 start=True, stop=True)
                nc.vector.tensor_add(out=hT[:, b * N:(b + 1) * N],
                                     in0=ps2[:], in1=hT[:, b * N:(b + 1) * N])
        bn_pass(hT, l, 0)

        # FF block (in place on hT)
        for (o, w) in CH:
            ffc = SCR.tile([128, 4, 512], F32, tag="ffc")
            for c in range(4):
                ps = PS.tile([128, 512], F32, tag="A")
                nc.tensor.matmul(ps[:, :w],
                                 lhsT=w_ff1[:, l, c * 128:(c + 1) * 128],
                                 rhs=hT[:, o:o + w], start=True, stop=True)
                nc.scalar.activation(out=ffc[:, c, :w], in_=ps[:, :w],
                                     func=AF.Relu, bias=b_ff1[:, l, c:c + 1],
                                     scale=1.0)
            ps2 = PS.tile([128, 512], F32, tag="A")
            for c in range(4):
                nc.tensor.matmul(ps2[:, :w], lhsT=w_ff2[:, l, c, :],
                                 rhs=ffc[:, c, :w],
                                 start=(c == 0), stop=(c == 3))
            nc.vector.scalar_tensor_tensor(
                out=hT[:, o:o + w], in0=ps2[:, :w], scalar=b_ff2[:, l:l + 1],
                in1=hT[:, o:o + w], op0=OP.add, op1=OP.add)
        bn_pass(hT, l, 1)

    if debug:
        nc.sync.dma_start(out=dd["ddbg"].ap(), in_=hT[:])

    # ---------------- decoder precompute ----------------
    hmean = SM.tile([D, BC], F32, tag="hmean")
    nc.vector.tensor_reduce(out=hmean[:],
                            in_=hT[:].rearrange("d (b n) -> d b n", b=BC),
                            axis=mybir.AxisListType.X, op=OP.add)
    nc.scalar.mul(out=hmean[:], in_=hmean[:], mul=1.0 / N)
    fixT = P.tile([D, BC], F32, tag="fixT")
    psf = PS.tile([D, BC], F32, tag="A")
    nc.tensor.matmul(psf[:], lhsT=w_fix[:], rhs=hmean[:], start=True, stop=True)
    nc.vector.tensor_copy(out=fixT[:], in_=psf[:])

    gKT = SCR.tile([D, BN], F32, tag="qT")
    lKT = SCR.tile([D, BN], F32, tag="kT")
    for blk, dst in ((0, gKT), (2, lKT)):
        for i, (o, w) in enumerate(CH):
            ps = PS.tile([128, 512], F32, tag="A")
            nc.tensor.matmul(ps[:, :w],
                             lhsT=w_nod[:, blk * 128:(blk + 1) * 128],
                             rhs=hT[:, o:o + w], start=True, stop=True)
            if i % 2 == 0:
                nc.scalar.copy(out=dst[:, o:o + w], in_=ps[:, :w])
            else:
                nc.vector.tensor_copy(out=dst[:, o:o + w], in_=ps[:, :w])
    gV_nat = SCR.tile([N, BC * D], F32, tag="v_nat")
    for b in range(BC):
        ps = PS.tile([N, 128], F32, tag="A")
        nc.tensor.matmul(ps[:], lhsT=hT[:, b * N:(b + 1) * N],
                         rhs=w_nod[:, 128:256], start=True, stop=True)
        if b % 2 == 0:
            nc.scalar.copy(out=gV_nat[:, b * D:(b + 1) * D], in_=ps[:])
        else:
            nc.vector.tensor_copy(out=gV_nat[:, b * D:(b + 1) * D], in_=ps[:])

    q0add = P.tile([D, 1], F32, tag="q0add")
    ps0 = PS.tile([D, 1], F32, tag="A")
    for c in range(2):
        nc.tensor.matmul(ps0[:], lhsT=w_step[:, c, :], rhs=wph[:, c:c + 1],
                         start=(c == 0), stop=(c == 1))
    nc.vector.tensor_copy(out=q0add[:], in_=ps0[:])

    # ---------------- PC tables: pc2 row b*101+p = (fix+W2 h[b,p]) . gK
    # (p=100 row: (fix + Wph@Wstep) . gK);  pc1 row b*100+p = (W1 h[b,p]) . gK
    q0T_all = P.tile([D, BC], F32, tag="q0T_all")
    nc.scalar.activation(out=q0T_all[:], in_=fixT[:], func=AF.Identity,
                         bias=q0add[:], scale=1.0)
    for b in range(BC):
        gkh = SCR.tile([128, 800], F32, tag="ffc")
        for h in range(H):
            nc.vector.tensor_scalar_mul(
                out=gkh[:, 100 * h:100 * h + 100],
                in0=gKT[:, b * N:(b + 1) * N], scalar1=hmask[:, h:h + 1])
        qs2 = SM.tile([128, 101], F32, tag="qs2")
        psq2 = PS.tile([128, 100], F32, tag="A")
        nc.tensor.matmul(psq2[:], lhsT=w_step[:, 1, :],
                         rhs=hT[:, b * N:(b + 1) * N], start=True, stop=True)
        nc.vector.tensor_scalar_add(out=qs2[:, :100], in0=psq2[:],
                                    scalar1=fixT[:, b:b + 1])
        nc.vector.tensor_copy(out=qs2[:, 100:101], in_=q0T_all[:, b:b + 1])
        psp2 = PS.tile([101, 800], F32, tag="A")
        for (o, w) in ((0, 512), (512, 288)):
            nc.tensor.matmul(psp2[:, o:o + w], lhsT=qs2[:],
                             rhs=gkh[:, o:o + w], start=True, stop=True,
                             skip_group_check=True)
        pcb2 = SCR.tile([101, 832], F32, tag="att_s")
        nc.vector.tensor_copy(out=pcb2[:, :800], in_=psp2[:])
        nc.vector.memset(pcb2[:, 800:832], 0.0)
        nc.sync.dma_start(out=dd["dpc2"].ap()[b * 101:(b + 1) * 101, :],
                          in_=pcb2[:])
        qs1 = SM.tile([128, 100], F32, tag="qs1")
        psq1 = PS.tile([128, 100], F32, tag="A")
        nc.tensor.matmul(psq1[:], lhsT=w_step[:, 0, :],
                         rhs=hT[:, b * N:(b + 1) * N], start=True, stop=True)
        nc.vector.tensor_copy(out=qs1[:], in_=psq1[:])
        psp1 = PS.tile([100, 800], F32, tag="A")
        for (o, w) in ((0, 512), (512, 288)):
            nc.tensor.matmul(psp1[:, o:o + w], lhsT=qs1[:],
                             rhs=gkh[:, o:o + w], start=True, stop=True,
                             skip_group_check=True)
        pcb1 = SCR.tile([101, 832], F32, tag="att_s")
        nc.vector.tensor_copy(out=pcb1[:100, :800], in_=psp1[:])
        nc.vector.memset(pcb1[:100, 800:832], 0.0)
        nc.sync.dma_start(out=dd["dpc1"].ap()[b * N:(b + 1) * N, :],
                          in_=pcb1[:100, :])

    # ---------------- decode state ----------------
    maskL = P.tile([BC, N], F32, tag="maskL")
    nc.vector.memset(maskL, 0.0)
    ll_acc = P.tile([BC, 1], F32, tag="ll_acc")
    nc.vector.memset(ll_acc, 0.0)
    pi_sb = P.tile([BC, N], U32, tag="pi_sb")
    att_sb = P.tile([BC, H * NPAD], F32, tag="att_d")
    nc.vector.memset(att_sb, 0.0)
    attT2 = P.tile([128, H * BC], F32, tag="attT2")
    G_sb = P.tile([128, 16 * 128], F32, tag="G_sb2")
    G_T = P.tile([128, 16 * 128], F32, tag="G_T2")
    gpT = P.tile([D, BC], F32, tag="gpT")
    base1 = P.tile([128, 832], F32, tag="base1")
    nc.vector.memset(base1, 0.0)
    sume_hist = P.tile([BC, N], F32, tag="sume_hist")
    sel_f = P.tile([BC, 1], F32, tag="sel_f")
    nc.vector.memset(sel_f, float(N))  # step 0 gathers the p=100 row of pc2
    gi_dyn = P.tile([128, 4], I16, tag="gi_dyn")
    boffg_i = P.tile([BC, 1], I32, tag="boffg_i")
    nc.gpsimd.iota(boffg_i, pattern=[[1, 1]], base=0, channel_multiplier=101)
    boffg_f = P.tile([BC, 1], F32, tag="boffg_f")
    nc.vector.tensor_copy(out=boffg_f[:], in_=boffg_i[:])

    def wrap_gather(idx_col, dst_sb, table, nrows):
        """idx_col [BC,1] f32 row-ids -> wrapped i16 -> dma_gather rows of
        `table` (DRAM [nrows, 800]) into dst_sb rows 0..BC."""
        psw = PS.tile([16, 4], F32, tag="A")
        for s in range(4):
            nc.tensor.matmul(psw[:, s:s + 1],
                             lhsT=ident[:BC, 16 * s:16 * s + 16],
                             rhs=idx_col[:], start=True, stop=True,
                             skip_group_check=True)
        wrap16f = SM.tile([16, 4], F32, tag="wrap16f")
        nc.vector.tensor_copy(out=wrap16f[:], in_=psw[:])
        psr = PS.tile([128, 4], F32, tag="A")
        nc.tensor.matmul(psr[:], lhsT=e16[:], rhs=wrap16f[:],
                         start=True, stop=True)
        nc.vector.tensor_copy(out=gi_dyn[:], in_=psr[:])
        nc.gpsimd.dma_gather(
            out_ap=dst_sb[:].unsqueeze(1), in_ap=table.ap(),
            idxs_ap=gi_dyn[:], num_idxs=BC, num_idxs_reg=BC, elem_size=832)

    def decode_step(i):
        # ---- gather compat from the PC tables ----
        idxg = SM.tile([BC, 1], F32, tag="idxg")
        nc.vector.tensor_add(out=idxg[:], in0=sel_f[:], in1=boffg_f[:])
        cmp_sb = SCR.tile([128, 832], F32, tag="cmp_sb")
        wrap_gather(idxg, cmp_sb, dd["dpc2"], BC * 101)
        if i == 1:
            # base1 = (W1 h[b, first]) . gK, fixed for the rest of the decode
            idxb = SM.tile([BC, 1], F32, tag="idxb")
            nc.vector.tensor_add(out=idxb[:], in0=sel_f[:], in1=boffc_f[:])
            wrap_gather(idxb, base1, dd["dpc1"], BC * N)
        # compat = gather + base1 + mask (broadcast over heads)
        nc.vector.tensor_add(out=cmp_sb[:BC, :800], in0=cmp_sb[:BC, :800],
                             in1=base1[:BC, :800])
        nc.vector.scalar_tensor_tensor(
            out=cmp_sb[:BC, :800].rearrange("b (h n) -> b h n", h=H),
            in0=cmp_sb[:BC, :800].rearrange("b (h n) -> b h n", h=H), scalar=1.0,
            in1=maskL[:].unsqueeze(1).broadcast_to([BC, H, N]),
            op0=OP.mult, op1=OP.add)
        # ---- softmax (unnormalized exp + per-head sums) ----
        nc.scalar.activation(
            out=att_sb[:].rearrange("b (h n) -> b h n", n=NPAD)[:, :, :N],
            in_=cmp_sb[:BC, :800].rearrange("b (h n) -> b h n", n=N),
            func=AF.Exp, scale=NF)
        s8 = SM.tile([BC, H], F32, tag="s8")
        nc.vector.tensor_reduce(
            out=s8[:], in_=att_sb[:].rearrange("b (h n) -> b h n", n=NPAD),
            axis=mybir.AxisListType.X, op=OP.add)
        nc.vector.reciprocal(out=s8[:], in_=s8[:])
        nc.vector.scalar_tensor_tensor(
            out=att_sb[:].rearrange("b (h n) -> b h n", n=NPAD),
            in0=att_sb[:].rearrange("b (h n) -> b h n", n=NPAD),
            scalar=1.0, in1=s8[:].unsqueeze(2).broadcast_to([BC, H, NPAD]),
            op0=OP.mult, op1=OP.mult)
        # ---- transpose att: 8 blocks [BC, 128] -> attT2 [128, (h, BC)] ----
        for hg in range(2):
            pst = PS.tile([128, 4 * BC], F32, tag="A")
            for hh in range(4):
                h = 4 * hg + hh
                nc.tensor.transpose(pst[:, hh * BC:(hh + 1) * BC],
                                    att_sb[:, h * NPAD:(h + 1) * NPAD],
                                    ident[:BC, :BC])
            # write in (quad, h, j) layout: col = 32*quad + 4*h + j, b = 4q+j
            dst = attT2[:].rearrange("p (q hh j) -> p q hh j", q=16,
                                     hh=H)[:, :, 4 * hg:4 * hg + 4, :]
            src_ap = pst[:].rearrange("p (hh q j) -> p q hh j", hh=4, q=16)
            if hg == 0:
                nc.scalar.copy(out=dst, in_=src_ap)
            else:
                nc.vector.tensor_copy(out=dst, in_=src_ap)
        # ---- glimpse: 16 quad matmuls (rhs = 4 samples' gV at once) ----
        for t in range(4):
            psg = PS.tile([128, 512], F32, tag="A")
            for qm in range(4):
                q = 4 * t + qm
                lhsT = attT2[:N, 32 * q:32 * q + 32]
                nc.tensor.matmul(
                    psg[32 * qm:32 * qm + 32, :],
                    lhsT=lhsT, rhs=gV_nat[:, 4 * q * D:(4 * q + 4) * D],
                    start=True, stop=True, tile_position=(0, 32 * qm),
                    skip_group_check=True)
            if t % 2 == 0:
                nc.scalar.copy(out=G_sb[:, t * 512:(t + 1) * 512], in_=psg[:])
            else:
                nc.vector.tensor_copy(out=G_sb[:, t * 512:(t + 1) * 512],
                                      in_=psg[:])
        # ---- transpose G (16 blocks) + diag-gather + out-proj ----
        for bgrp in range(4):
            pst = PS.tile([128, 512], F32, tag="A")
            for bb in range(4):
                blk = 4 * bgrp + bb
                nc.tensor.transpose(pst[:, bb * 128:(bb + 1) * 128],
                                    G_sb[:, blk * 128:(blk + 1) * 128],
                                    ident[:])
            if bgrp % 2 == 0:
                nc.scalar.copy(out=G_T[:, bgrp * 512:(bgrp + 1) * 512],
                               in_=pst[:])
            else:
                nc.vector.tensor_copy(out=G_T[:, bgrp * 512:(bgrp + 1) * 512],
                                      in_=pst[:])
        glT = SM.tile([128, BC], F32, tag="glT")
        nc.gpsimd.ap_gather(
            out_ap=glT[:].unsqueeze(2), in_ap=G_T[:].unsqueeze(2),
            idxs_ap=gidx_sb[:], channels=128, num_elems=16 * 128, d=1,
            num_idxs=BC)
        psp = PS.tile([D, BC], F32, tag="A")
        nc.tensor.matmul(psp[:], lhsT=w_outd[:], rhs=glT[:], start=True,
                         stop=True)
        nc.vector.tensor_copy(out=gpT[:], in_=psp[:])
        # ---- logits ----
        psl = PS.tile([N, BC], F32, tag="A")
        for b in range(BC):
            nc.tensor.matmul(psl[:, b:b + 1], lhsT=lKT[:, b * N:(b + 1) * N],
                             rhs=gpT[:, b:b + 1], start=True, stop=True,
                             skip_group_check=True)
        lT_sb = SM.tile([N, BC], F32, tag="lT_sb")
        nc.vector.tensor_copy(out=lT_sb[:], in_=psl[:])
        psl2 = PS.tile([BC, N], F32, tag="A")
        nc.tensor.transpose(psl2[:], lT_sb[:], ident[:N, :N])
        logits = SM.tile([BC, N], F32, tag="logits")
        nc.scalar.activation(out=logits[:], in_=psl2[:], func=AF.Tanh, scale=NF)
        nc.vector.scalar_tensor_tensor(
            out=logits[:], in0=logits[:], scalar=CLIP, in1=maskL[:],
            op0=OP.mult, op1=OP.add)
        # ---- argmax / lse / ll ----
        if debug and i == 0:
            nc.sync.dma_start(out=dd["ddbg_l"].ap(), in_=logits[:])
            nc.sync.dma_start(out=dd["ddbg_g"].ap(), in_=gpT[:])
        mx8 = SM.tile([BC, 8], F32, tag="mx8")
        ix8 = SM.tile([BC, 8], U32, tag="ix8")
        nc.vector.max_with_indices(mx8[:], ix8[:], logits[:])
        negmx = SM.tile([BC, 1], F32, tag="negmx")
        nc.vector.tensor_scalar_mul(out=negmx[:], in0=mx8[:, 0:1], scalar1=-1.0)
        esc = SM.tile([BC, N], F32, tag="esc")
        nc.scalar.activation(out=esc[:], in_=logits[:], func=AF.Exp,
                             bias=negmx[:], scale=1.0,
                             accum_out=sume_hist[:, i:i + 1])
        nc.vector.tensor_copy(out=sel_f[:], in_=ix8[:, 0:1])
        nc.vector.tensor_copy(out=pi_sb[:, i:i + 1], in_=ix8[:, 0:1])
        # ---- mask update ----
        onel = SM.tile([BC, N], F32, tag="onel")
        nc.vector.tensor_scalar(
            out=onel[:], in0=iota_f[:BC, :], scalar1=sel_f[:],
            scalar2=float(NEG), op0=OP.is_equal, op1=OP.mult)
        nc.vector.tensor_add(out=maskL[:], in0=maskL[:], in1=onel[:])

    for i in range(N):
        decode_step(i)

    lnh = SM.tile([BC, N], F32, tag="esc")
    nc.scalar.activation(out=lnh[:], in_=sume_hist[:], func=AF.Ln)
    nc.vector.tensor_reduce(out=ll_acc[:], in_=lnh[:],
                            axis=mybir.AxisListType.X, op=OP.add)
    nc.vector.tensor_scalar_mul(out=ll_acc[:], in0=ll_acc[:], scalar1=-1.0)
    nc.sync.dma_start(out=dll.ap().rearrange("(b o) -> b o", o=1), in_=ll_acc[:])
    pi_i32 = SM.tile([BC, N], I32, tag="pi_i32")
    nc.vector.tensor_copy(out=pi_i32[:], in_=pi_sb[:])
    nc.sync.dma_start(out=dpi.ap(), in_=pi_i32[:])


def make_e4():
    t = np.zeros((4, 128), np.float32)
    for s in range(4):
        t[s, 32 * s:32 * s + 8] = NEG
    return t


def make_e16():
    t = np.zeros((16, 128), np.float32)
    for g in range(8):
        for p in range(16):
            t[p, 16 * g + p] = 1.0
    return t


def make_hmask():
    t = np.zeros((128, 8), np.float32)
    for h in range(8):
        t[16 * h:16 * h + 16, h] = 1.0
    return t


def make_gidx():
    tbl = np.zeros((128, BC // 16), np.int16)
    for grp in range(8):
        for i in range(BC):
            p, slot = i % 16, i // 16
            t, qm, j = i // 16, (i % 16) // 4, i % 4
            tbl[16 * grp + p, slot] = 128 * (4 * t + j) + 32 * qm + 4 * grp + j
    return tbl


_CACHE = {}


def get_nc(debug=False):
    key = bool(debug)
    if key not in _CACHE:
        _CACHE[key] = build_nc(debug=debug)
    return _CACHE[key]


def host_cost(x, pi):
    d = np.take_along_axis(x, np.broadcast_to(pi[:, :, None],
                                              (pi.shape[0], N, 2)), 1)
    return (np.linalg.norm(d[:, 1:] - d[:, :-1], axis=-1).sum(1)
            + np.linalg.norm(d[:, 0] - d[:, -1], axis=-1)).astype(np.float32)


def kernel(trace=False, **inputs):
    nc = get_nc(debug=False)
    gidx = make_gidx()
    x_full = np.ascontiguousarray(np.asarray(inputs["x"], np.float32))
    in_maps = []
    for c in range(NCORE):
        m = {k: np.ascontiguousarray(np.asarray(v, dtype=np.float32))
             for k, v in inputs.items() if k != "x"}
        m["x"] = np.ascontiguousarray(x_full[c * BC:(c + 1) * BC])
        m["gidx"] = gidx
        m["e4c"] = make_e4()
        m["hmask"] = make_hmask()
        m["e16c"] = make_e16()
        in_maps.append(m)
    res = run_bass_kernel_spmd(nc, in_maps, core_ids=list(range(NCORE)),
                               trace=trace)
    if trace:
        print("exec_time_ns:", res.exec_time_ns)
        print("trace:", res.instructions_and_trace[1]
              if res.instructions_and_trace else None)
        print("profile_json:", res.profile_json)
        import json
        with open("/root/problem/trace_info.json", "w") as f:
            json.dump({"exec_time_ns": res.exec_time_ns,
                       "profile_json": res.profile_json,
                       "trace": res.instructions_and_trace[1]
                       if res.instructions_and_trace else None}, f)
    ll = np.concatenate([r["ll"] for r in res.results])
    pi = np.concatenate([r["pi"] for r in res.results]).astype(np.int32)
    cost = host_cost(x_full, pi)
    return cost, ll, pi


# revision 45
# speedup vs baseline: 1.2861x; 1.2861x over previous
"""Trainium2 Bass kernel for nn_AttentionModel (Kool-style TSP attention model).

Data-parallel over 8 NeuronCores: each core processes B/8 = 64 samples,
fp32 throughout, transposed activation layouts (features on partitions).
The TSP tour cost is reconstructed on the host from pi (pure indexing).
"""

import numpy as np

import concourse.bass as bass
import concourse.mybir as mybir
import concourse.tile as tile
from concourse import bacc
from concourse.bass_utils import run_bass_kernel_spmd
from concourse.masks import make_identity

F32 = mybir.dt.float32
I32 = mybir.dt.int32
U32 = mybir.dt.uint32
I16 = mybir.dt.int16
AF = mybir.ActivationFunctionType
OP = mybir.AluOpType

B, N, D, H, L, FF = 512, 100, 128, 8, 2, 512
DK = D // H          # 16
NCORE = 8
BC = B // NCORE      # 64 samples per core
BN = BC * N          # 6400
NPAD = 128
CLIP = 10.0
NEG = -1e9
NF = 1.0 / float(np.sqrt(D))      # decoder norm factor
SC = 1.0 / float(np.sqrt(DK))     # encoder attention scale
EPS = 1e-5

CH = [(o, min(512, BN - o)) for o in range(0, BN, 512)]


def build_nc(debug=False, use_cc=True):
    nc = bacc.Bacc("TRN2", target_bir_lowering=False, debug=False,
                   num_devices=NCORE if use_cc else 1)

    dx = nc.dram_tensor("x", [BC, N, 2], F32, kind="ExternalInput")
    dinit_W = nc.dram_tensor("init_W", [2, D], F32, kind="ExternalInput")
    dinit_b = nc.dram_tensor("init_b", [D], F32, kind="ExternalInput")
    dqkv = nc.dram_tensor("enc_qkv_W", [L, D, 3 * D], F32, kind="ExternalInput")
    doutW = nc.dram_tensor("enc_out_W", [L, D, D], F32, kind="ExternalInput")
    dbn1 = nc.dram_tensor("enc_bn1", [L, 2, D], F32, kind="ExternalInput")
    dbn2 = nc.dram_tensor("enc_bn2", [L, 2, D], F32, kind="ExternalInput")
    dff1W = nc.dram_tensor("enc_ff1_W", [L, D, FF], F32, kind="ExternalInput")
    dff1b = nc.dram_tensor("enc_ff1_b", [L, FF], F32, kind="ExternalInput")
    dff2W = nc.dram_tensor("enc_ff2_W", [L, FF, D], F32, kind="ExternalInput")
    dff2b = nc.dram_tensor("enc_ff2_b", [L, D], F32, kind="ExternalInput")
    dWph = nc.dram_tensor("W_placeholder", [2 * D], F32, kind="ExternalInput")
    dWnod = nc.dram_tensor("proj_nodes_W", [D, 3 * D], F32, kind="ExternalInput")
    dWfix = nc.dram_tensor("proj_fixed_W", [D, D], F32, kind="ExternalInput")
    dWstep = nc.dram_tensor("proj_step_W", [2 * D, D], F32, kind="ExternalInput")
    dWout = nc.dram_tensor("proj_out_W", [D, D], F32, kind="ExternalInput")
    dgidx = nc.dram_tensor("gidx", [128, BC // 16], I16, kind="ExternalInput")
    de4 = nc.dram_tensor("e4c", [4, 128], F32, kind="ExternalInput")
    dhm = nc.dram_tensor("hmask", [128, 8], F32, kind="ExternalInput")
    de16 = nc.dram_tensor("e16c", [16, 128], F32, kind="ExternalInput")

    dll = nc.dram_tensor("ll", [BC], F32, kind="ExternalOutput")
    dpi = nc.dram_tensor("pi", [BC, N], I32, kind="ExternalOutput")
    if debug:
        ddbg = nc.dram_tensor("dbg_h", [D, BN], F32, kind="ExternalOutput")
        ddbg_l = nc.dram_tensor("dbg_l", [BC, N], F32, kind="ExternalOutput")
        ddbg_g = nc.dram_tensor("dbg_g", [D, BC], F32, kind="ExternalOutput")
        ddbg_a = nc.dram_tensor("dbg_a", [128, 16 * NPAD], F32,
                                kind="ExternalOutput")
    dpc2 = nc.dram_tensor("pc2", [BC * 101, 832], F32)
    dpc1 = nc.dram_tensor("pc1", [BC * N, 832], F32)
    cc_bufs = []
    if use_cc:
        for i in range(2 * L):
            cc_bufs.append((nc.dram_tensor(f"ccin{i}", [D, 2], F32),
                            nc.dram_tensor(f"ccout{i}", [D, 2], F32)))

    with tile.TileContext(nc) as tc:
        with tc.tile_pool(name="P", bufs=1) as P, \
             tc.tile_pool(name="SCR", bufs=1) as SCR, \
             tc.tile_pool(name="SM", bufs=2) as SM, \
             tc.tile_pool(name="PS", bufs=3, space="PSUM") as PS:
            _build_model(nc, tc, P, SCR, SM, PS, locals(), debug, use_cc,
                         cc_bufs)
    nc.compile()
    return nc


def _build_model(nc, tc, P, SCR, SM, PS, dd, debug, use_cc, cc_bufs):
    dx, dinit_W, dinit_b = dd["dx"], dd["dinit_W"], dd["dinit_b"]
    dqkv, doutW, dbn1, dbn2 = dd["dqkv"], dd["doutW"], dd["dbn1"], dd["dbn2"]
    dff1W, dff1b, dff2W, dff2b = dd["dff1W"], dd["dff1b"], dd["dff2W"], dd["dff2b"]
    dWph, dWnod, dWfix, dWstep, dWout = (dd["dWph"], dd["dWnod"], dd["dWfix"],
                                         dd["dWstep"], dd["dWout"])
    dgidx, dll, dpi = dd["dgidx"], dd["dll"], dd["dpi"]

    # ---------------- weights ----------------
    ident = P.tile([128, 128], F32, tag="ident")
    make_identity(nc, ident)

    w_init = P.tile([2, D], F32, tag="w_init")
    nc.sync.dma_start(out=w_init, in_=dinit_W.ap())
    b_init = P.tile([D, 1], F32, tag="b_init")
    nc.sync.dma_start(out=b_init, in_=dinit_b.ap().rearrange("(d o) -> d o", o=1))
    w_qkv = P.tile([D, L, 3 * D], F32, tag="w_qkv")
    nc.sync.dma_start(out=w_qkv[:], in_=dqkv.ap().transpose([1, 0, 2]))
    w_out = P.tile([D, L, D], F32, tag="w_out")
    nc.sync.dma_start(out=w_out[:], in_=doutW.ap().transpose([1, 0, 2]))
    w_ff1 = P.tile([D, L, FF], F32, tag="w_ff1")
    nc.sync.dma_start(out=w_ff1[:], in_=dff1W.ap().transpose([1, 0, 2]))
    b_ff1 = P.tile([128, L, FF // 128], F32, tag="b_ff1")
    nc.sync.dma_start(
        out=b_ff1[:],
        in_=dff1b.ap().rearrange("l (c p) -> p l c", p=128))
    w_ff2 = P.tile([128, L, FF // 128, D], F32, tag="w_ff2")
    nc.sync.dma_start(
        out=w_ff2[:],
        in_=dff2W.ap().rearrange("l (c p) d -> p l c d", p=128))
    b_ff2 = P.tile([D, L], F32, tag="b_ff2")
    nc.sync.dma_start(out=b_ff2[:], in_=dff2b.ap().transpose([1, 0]))
    bn_gb1 = P.tile([D, L, 2], F32, tag="bn_gb1")  # [d, l, gamma/beta]
    nc.sync.dma_start(out=bn_gb1[:], in_=dbn1.ap().transpose([2, 0, 1]))
    bn_gb2 = P.tile([D, L, 2], F32, tag="bn_gb2")
    nc.sync.dma_start(out=bn_gb2[:], in_=dbn2.ap().transpose([2, 0, 1]))
    w_nod = P.tile([D, 3 * D], F32, tag="w_nod")
    nc.sync.dma_start(out=w_nod[:], in_=dWnod.ap())
    w_fix = P.tile([D, D], F32, tag="w_fix")
    nc.sync.dma_start(out=w_fix[:], in_=dWfix.ap())
    w_step = P.tile([128, 2, D], F32, tag="w_step")
    nc.sync.dma_start(out=w_step[:],
                      in_=dWstep.ap().rearrange("(c p) d -> p c d", p=128))
    w_outd = P.tile([D, D], F32, tag="w_outd")
    nc.sync.dma_start(out=w_outd[:], in_=dWout.ap())
    wph = P.tile([128, 2], F32, tag="wph")
    nc.sync.dma_start(out=wph[:], in_=dWph.ap().rearrange("(c p) -> p c", p=128))
    gidx_sb = P.tile([128, BC // 16], I16, tag="gidx")
    nc.sync.dma_start(out=gidx_sb[:], in_=dgidx.ap())

    # mask injector: e4[s, 32s+h] = NEG for h<8 (host-provided)
    e4 = P.tile([4, 128], F32, tag="e4")
    nc.sync.dma_start(out=e4[:], in_=dd["de4"].ap())
    hmask = P.tile([128, 8], F32, tag="hmask")
    nc.sync.dma_start(out=hmask[:], in_=dd["dhm"].ap())
    e16 = P.tile([16, 128], F32, tag="e16")
    nc.sync.dma_start(out=e16[:], in_=dd["de16"].ap())
    iota_n = P.tile([64, N], I32, tag="iota_n")
    nc.gpsimd.iota(iota_n, pattern=[[1, N]], base=0, channel_multiplier=0)
    iota_f = P.tile([64, N], F32, tag="iota_f")
    nc.vector.tensor_copy(out=iota_f[:], in_=iota_n[:])
    # row of b*100 offsets [1, 64]
    boff_i = P.tile([1, BC], I32, tag="boff_i")
    nc.gpsimd.iota(boff_i, pattern=[[N, BC]], base=0, channel_multiplier=0)
    boff_f = P.tile([1, BC], F32, tag="boff_f")
    nc.vector.tensor_copy(out=boff_f[:], in_=boff_i[:])
    boffc_i = P.tile([BC, 1], I32, tag="boffc_i")
    nc.gpsimd.iota(boffc_i, pattern=[[1, 1]], base=0, channel_multiplier=N)
    boffc_f = P.tile([BC, 1], F32, tag="boffc_f")
    nc.vector.tensor_copy(out=boffc_f[:], in_=boffc_i[:])

    eps_t = P.tile([D, 1], F32, tag="eps_t")
    nc.vector.memset(eps_t, EPS)

    hT = P.tile([D, BN], F32, tag="hT")

    # ---------------- h0 = (x @ init_W + init_b)^T ----------------
    for (o, w) in CH:
        xTs = SCR.tile([2, 512], F32, tag="xm")
        nc.sync.dma_start(
            out=xTs[:, :w],
            in_=bass.AP(tensor=dx.ap().tensor, offset=2 * o,
                        ap=[[1, 2], [2, w]]))
        ps = PS.tile([128, 512], F32, tag="A")
        nc.tensor.matmul(ps[:, :w], lhsT=w_init[:], rhs=xTs[:, :w],
                         start=True, stop=True)
        nc.scalar.activation(out=hT[:, o:o + w], in_=ps[:, :w],
                             func=AF.Identity, bias=b_init[:], scale=1.0)

    # ---------------- BatchNorm helper (in place on [D, BN]) ----------------
    def bn_pass(t, l, which):
        nsub = BN // 128  # 50 subgroups of 128 for bn_stats
        stats = SM.tile([D, nsub, 6], F32, tag="bnstats")
        tv = t[:].rearrange("d (s c) -> d s c", c=128)
        for s in range(nsub):
            nc.vector.bn_stats(out=stats[:, s, :], in_=tv[:, s, :])
        mv = SM.tile([D, 2], F32, tag="bnmv")
        nc.vector.bn_aggr(out=mv[:], in_=stats[:])
        if use_cc:
            # cross-core stats: allreduce (mean, var + mean^2), divide by 8
            pay = SM.tile([D, 2], F32, tag="ccpay")
            nc.vector.tensor_copy(out=pay[:, 0:1], in_=mv[:, 0:1])
            nc.vector.scalar_tensor_tensor(
                out=pay[:, 1:2], in0=mv[:, 0:1], scalar=mv[:, 0:1],
                in1=mv[:, 1:2], op0=OP.mult, op1=OP.add)
            cin, cout = cc_bufs[2 * l + which]
            nc.sync.dma_start(out=cin.ap(), in_=pay[:])
            nc.gpsimd.collective_compute(
                "AllReduce", OP.add, replica_groups=[list(range(NCORE))],
                ins=[cin.ap()], outs=[cout.ap()])
            nc.sync.dma_start(out=pay[:], in_=cout.ap())
            # mean = pay0/8 ; var = pay1/8 - mean^2
            nc.scalar.mul(out=mv[:, 0:1], in_=pay[:, 0:1], mul=1.0 / NCORE)
            msq = SM.tile([D, 1], F32, tag="ccmsq")
            nc.vector.tensor_mul(out=msq[:], in0=mv[:, 0:1], in1=mv[:, 0:1])
            nc.vector.tensor_scalar(
                out=mv[:, 1:2], in0=pay[:, 1:2], scalar1=1.0 / NCORE,
                scalar2=msq[:], op0=OP.mult, op1=OP.subtract)
        rstd = SM.tile([D, 1], F32, tag="bnrstd")
        nc.scalar.activation(out=rstd[:], in_=mv[:, 1:2], func=AF.Sqrt,
                             bias=eps_t[:], scale=1.0)
        nc.vector.reciprocal(out=rstd[:], in_=rstd[:])
        scale = SM.tile([D, 1], F32, tag="bnscale")
        nc.vector.tensor_mul(out=scale[:], in0=rstd[:],
                             in1=(bn_gb1 if which == 0 else bn_gb2)[:, l, 0:1])
        shift = SM.tile([D, 1], F32, tag="bnshift")
        nc.vector.tensor_mul(out=shift[:], in0=mv[:, 0:1], in1=scale[:])
        nc.vector.tensor_scalar(
            out=shift[:], in0=(bn_gb1 if which == 0 else bn_gb2)[:, l, 1:2], scalar1=shift[:],
            scalar2=None, op0=OP.subtract)
        for i, (o, w) in enumerate(CH):
            if i % 2 == 0:
                nc.scalar.activation(out=t[:, o:o + w], in_=t[:, o:o + w],
                                     func=AF.Identity, bias=shift[:],
                                     scale=scale[:])
            else:
                nc.vector.tensor_scalar(
                    out=t[:, o:o + w], in0=t[:, o:o + w], scalar1=scale[:],
                    scalar2=shift[:], op0=OP.mult, op1=OP.add)

    # ---------------- encoder ----------------
    for l in range(L):
        qT = SCR.tile([D, BN], F32, tag="qT")
        kT = SCR.tile([D, BN], F32, tag="kT")
        for blk, dst in ((0, qT), (1, kT)):
            for i, (o, w) in enumerate(CH):
                ps = PS.tile([128, 512], F32, tag="A")
                nc.tensor.matmul(ps[:, :w],
                                 lhsT=w_qkv[:, l, blk * 128:(blk + 1) * 128],
                                 rhs=hT[:, o:o + w], start=True, stop=True)
                if i % 2 == 0:
                    nc.scalar.copy(out=dst[:, o:o + w], in_=ps[:, :w])
                else:
                    nc.vector.tensor_copy(out=dst[:, o:o + w], in_=ps[:, :w])
        v_nat = SCR.tile([N, BC * D], F32, tag="v_nat")
        for b in range(BC):
            ps = PS.tile([N, 128], F32, tag="A")
            nc.tensor.matmul(ps[:], lhsT=hT[:, b * N:(b + 1) * N],
                             rhs=w_qkv[:, l, 256:384], start=True, stop=True)
            if b % 2 == 0:
                nc.scalar.copy(out=v_nat[:, b * D:(b + 1) * D], in_=ps[:])
            else:
                nc.vector.tensor_copy(out=v_nat[:, b * D:(b + 1) * D], in_=ps[:])

        # attention in groups of 4 samples (zero-embedded khat)
        GS = 2
        khat = SCR.tile([128, GS * 800], F32, tag="khat")
        nc.vector.memset(khat, 0.0)
        for bg in range(BC // GS):
            b0 = bg * GS
            for h in range(H):
                nc.vector.tensor_scalar_mul(
                    out=khat[:, :].rearrange(
                        "p (bb m) -> p bb m", bb=GS)[:, :, 100 * h:100 * h + 100],
                    in0=kT[:, b0 * N:(b0 + GS) * N].rearrange(
                        "p (bb m) -> p bb m", bb=GS),
                    scalar1=hmask[:, h:h + 1])
            att_s = SCR.tile([N, GS * 800], F32, tag="att_s")
            for bb in range(GS):
                b = b0 + bb
                ps = PS.tile([N, 800], F32, tag="A")
                for (o, w) in ((0, 512), (512, 288)):
                    nc.tensor.matmul(
                        ps[:, o:o + w], lhsT=qT[:, b * N:(b + 1) * N],
                        rhs=khat[:, bb * 800 + o: bb * 800 + o + w],
                        start=True, stop=True, skip_group_check=True)
                nc.scalar.activation(out=att_s[:, bb * 800:(bb + 1) * 800],
                                     in_=ps[:], func=AF.Exp, scale=SC)
            ssum = SM.tile([N, GS * 8], F32, tag="ssum")
            nc.vector.tensor_reduce(
                out=ssum[:],
                in_=att_s[:].rearrange("p (bb h m) -> p (bb h) m", bb=GS, h=H),
                axis=mybir.AxisListType.X, op=OP.add)
            nc.vector.reciprocal(out=ssum[:], in_=ssum[:])
            nc.vector.scalar_tensor_tensor(
                out=att_s[:].rearrange("p (bb h m) -> p (bb h) m", bb=GS, h=H),
                in0=att_s[:].rearrange("p (bb h m) -> p (bb h) m", bb=GS, h=H),
                scalar=1.0,
                in1=ssum[:].unsqueeze(2).broadcast_to([N, GS * 8, 100]),
                op0=OP.mult, op1=OP.mult)
            # transpose att blocks (batched per 4 heads); o = att @ v
            o_all = SCR.tile([N, GS * D], F32, tag="o_all")
            for bb in range(GS):
                b = b0 + bb
                atT = SCR.tile([100, 800], F32, tag="xm")
                for hg in range(2):
                    pst = PS.tile([128, 400], F32, tag="A")
                    for hh in range(4):
                        h = 4 * hg + hh
                        nc.tensor.transpose(
                            pst[:100, hh * 100:(hh + 1) * 100],
                            att_s[:, bb * 800 + h * 100: bb * 800 + (h + 1) * 100],
                            ident[:100, :100])
                    if hg == 0:
                        nc.scalar.copy(out=atT[:, :400], in_=pst[:100, :])
                    else:
                        nc.vector.tensor_copy(out=atT[:, 400:], in_=pst[:100, :])
                pso = PS.tile([N, D], F32, tag="A")
                for h in range(H):
                    nc.tensor.matmul(
                        pso[:, h * DK:(h + 1) * DK],
                        lhsT=atT[:, h * 100:(h + 1) * 100],
                        rhs=v_nat[:, b * D + h * DK: b * D + (h + 1) * DK],
                        start=True, stop=True, skip_group_check=True)
                nc.vector.tensor_copy(out=o_all[:, bb * D:(bb + 1) * D],
                                      in_=pso[:])
            # transpose o per sample and project + residual into hT
            for bb in range(GS):
                b = b0 + bb
                pst = PS.tile([128, 100], F32, tag="A")
                nc.tensor.transpose(pst[:], o_all[:, bb * D:(bb + 1) * D],
                                    ident[:100, :100])
                oTs = SM.tile([128, 100], F32, tag="oTs")
                nc.vector.tensor_copy(out=oTs[:], in_=pst[:])
                ps2 = PS.tile([128, 100], F32, tag="A")
                nc.tensor.matmul(ps2[:], lhsT=w_out[:, l, :], rhs=oTs[:],
                                 start=True, stop=True)
                nc.vector.tensor_add(out=hT[:, b * N:(b + 1) * N],
                                     in0=ps2[:], in1=hT[:, b * N:(b + 1) * N])
        bn_pass(hT, l, 0)

        # FF block (in place on hT)
        for (o, w) in CH:
            ffc = SCR.tile([128, 4, 512], F32, tag="ffc")
            for c in range(4):
                ps = PS.tile([128, 512], F32, tag="A")
                nc.tensor.matmul(ps[:, :w],
                                 lhsT=w_ff1[:, l, c * 128:(c + 1) * 128],
                                 rhs=hT[:, o:o + w], start=True, stop=True)
                nc.scalar.activation(out=ffc[:, c, :w], in_=ps[:, :w],
                                     func=AF.Relu, bias=b_ff1[:, l, c:c + 1],
                                     scale=1.0)
            ps2 = PS.tile([128, 512], F32, tag="A")
            for c in range(4):
                nc.tensor.matmul(ps2[:, :w], lhsT=w_ff2[:, l, c, :],
                                 rhs=ffc[:, c, :w],
                                 start=(c == 0), stop=(c == 3))
            nc.vector.scalar_tensor_tensor(
                out=hT[:, o:o + w], in0=ps2[:, :w], scalar=b_ff2[:, l:l + 1],
                in1=hT[:, o:o + w], op0=OP.add, op1=OP.add)
        bn_pass(hT, l, 1)

    if debug:
        nc.sync.dma_start(out=dd["ddbg"].ap(), in_=hT[:])

    # ---------------- decoder precompute ----------------
    hmean = SM.tile([D, BC], F32, tag="hmean")
    nc.vector.tensor_reduce(out=hmean[:],
                            in_=hT[:].rearrange("d (b n) -> d b n", b=BC),
                            axis=mybir.AxisListType.X, op=OP.add)
    nc.scalar.mul(out=hmean[:], in_=hmean[:], mul=1.0 / N)
    fixT = P.tile([D, BC], F32, tag="fixT")
    psf = PS.tile([D, BC], F32, tag="A")
    nc.tensor.matmul(psf[:], lhsT=w_fix[:], rhs=hmean[:], start=True, stop=True)
    nc.vector.tensor_copy(out=fixT[:], in_=psf[:])

    gKT = SCR.tile([D, BN], F32, tag="qT")
    lKT = SCR.tile([D, BN], F32, tag="kT")
    for blk, dst in ((0, gKT), (2, lKT)):
        for i, (o, w) in enumerate(CH):
            ps = PS.tile([128, 512], F32, tag="A")
            nc.tensor.matmul(ps[:, :w],
                             lhsT=w_nod[:, blk * 128:(blk + 1) * 128],
                             rhs=hT[:, o:o + w], start=True, stop=True)
            if i % 2 == 0:
                nc.scalar.copy(out=dst[:, o:o + w], in_=ps[:, :w])
            else:
                nc.vector.tensor_copy(out=dst[:, o:o + w], in_=ps[:, :w])
    gV_nat = SCR.tile([N, BC * D], F32, tag="v_nat")
    for b in range(BC):
        ps = PS.tile([N, 128], F32, tag="A")
        nc.tensor.matmul(ps[:], lhsT=hT[:, b * N:(b + 1) * N],
                         rhs=w_nod[:, 128:256], start=True, stop=True)
        if b % 2 == 0:
            nc.scalar.copy(out=gV_nat[:, b * D:(b + 1) * D], in_=ps[:])
        else:
            nc.vector.tensor_copy(out=gV_nat[:, b * D:(b + 1) * D], in_=ps[:])

    q0add = P.tile([D, 1], F32, tag="q0add")
    ps0 = PS.tile([D, 1], F32, tag="A")
    for c in range(2):
        nc.tensor.matmul(ps0[:], lhsT=w_step[:, c, :], rhs=wph[:, c:c + 1],
                         start=(c == 0), stop=(c == 1))
    nc.vector.tensor_copy(out=q0add[:], in_=ps0[:])

    # ---------------- PC tables: pc2 row b*101+p = (fix+W2 h[b,p]) . gK
    # (p=100 row: (fix + Wph@Wstep) . gK);  pc1 row b*100+p = (W1 h[b,p]) . gK
    q0T_all = P.tile([D, BC], F32, tag="q0T_all")
    nc.scalar.activation(out=q0T_all[:], in_=fixT[:], func=AF.Identity,
                         bias=q0add[:], scale=1.0)
    for b in range(BC):
        gkh = SCR.tile([128, 800], F32, tag="ffc")
        for h in range(H):
            nc.vector.tensor_scalar_mul(
                out=gkh[:, 100 * h:100 * h + 100],
                in0=gKT[:, b * N:(b + 1) * N], scalar1=hmask[:, h:h + 1])
        qs2 = SM.tile([128, 101], F32, tag="qs2")
        psq2 = PS.tile([128, 100], F32, tag="A")
        nc.tensor.matmul(psq2[:], lhsT=w_step[:, 1, :],
                         rhs=hT[:, b * N:(b + 1) * N], start=True, stop=True)
        nc.vector.tensor_scalar_add(out=qs2[:, :100], in0=psq2[:],
                                    scalar1=fixT[:, b:b + 1])
        nc.vector.tensor_copy(out=qs2[:, 100:101], in_=q0T_all[:, b:b + 1])
        psp2 = PS.tile([101, 800], F32, tag="A")
        for (o, w) in ((0, 512), (512, 288)):
            nc.tensor.matmul(psp2[:, o:o + w], lhsT=qs2[:],
                             rhs=gkh[:, o:o + w], start=True, stop=True,
                             skip_group_check=True)
        pcb2 = SCR.tile([101, 832], F32, tag="att_s")
        nc.vector.tensor_copy(out=pcb2[:, :800], in_=psp2[:])
        nc.vector.memset(pcb2[:, 800:832], 0.0)
        nc.sync.dma_start(out=dd["dpc2"].ap()[b * 101:(b + 1) * 101, :],
                          in_=pcb2[:])
        qs1 = SM.tile([128, 100], F32, tag="qs1")
        psq1 = PS.tile([128, 100], F32, tag="A")
        nc.tensor.matmul(psq1[:], lhsT=w_step[:, 0, :],
                         rhs=hT[:, b * N:(b + 1) * N], start=True, stop=True)
        nc.vector.tensor_copy(out=qs1[:], in_=psq1[:])
        psp1 = PS.tile([100, 800], F32, tag="A")
        for (o, w) in ((0, 512), (512, 288)):
            nc.tensor.matmul(psp1[:, o:o + w], lhsT=qs1[:],
                             rhs=gkh[:, o:o + w], start=True, stop=True,
                             skip_group_check=True)
        pcb1 = SCR.tile([101, 832], F32, tag="att_s")
        nc.vector.tensor_copy(out=pcb1[:100, :800], in_=psp1[:])
        nc.vector.memset(pcb1[:100, 800:832], 0.0)
        nc.sync.dma_start(out=dd["dpc1"].ap()[b * N:(b + 1) * N, :],
                          in_=pcb1[:100, :])

    # ---------------- decode state ----------------
    maskL = P.tile([BC, N], F32, tag="maskL")
    nc.vector.memset(maskL, 0.0)
    ll_acc = P.tile([BC, 1], F32, tag="ll_acc")
    nc.vector.memset(ll_acc, 0.0)
    pi_sb = P.tile([BC, N], U32, tag="pi_sb")
    att_sb = P.tile([BC, H * NPAD], F32, tag="att_d")
    nc.vector.memset(att_sb, 0.0)
    attT2 = P.tile([128, H * BC], F32, tag="attT2")
    G_sb = P.tile([128, 16 * 128], F32, tag="G_sb2")
    G_T = P.tile([128, 16 * 128], F32, tag="G_T2")
    gpT = P.tile([D, BC], F32, tag="gpT")
    base1 = P.tile([128, 832], F32, tag="base1")
    nc.vector.memset(base1, 0.0)
    sume_hist = P.tile([BC, N], F32, tag="sume_hist")
    sel_f = P.tile([BC, 1], F32, tag="sel_f")
    nc.vector.memset(sel_f, float(N))  # step 0 gathers the p=100 row of pc2
    gi_dyn = P.tile([128, 4], I16, tag="gi_dyn")
    boffg_i = P.tile([BC, 1], I32, tag="boffg_i")
    nc.gpsimd.iota(boffg_i, pattern=[[1, 1]], base=0, channel_multiplier=101)
    boffg_f = P.tile([BC, 1], F32, tag="boffg_f")
    nc.vector.tensor_copy(out=boffg_f[:], in_=boffg_i[:])

    def wrap_gather(idx_col, dst_sb, table, nrows):
        """idx_col [BC,1] f32 row-ids -> wrapped i16 -> dma_gather rows of
        `table` (DRAM [nrows, 800]) into dst_sb rows 0..BC."""
        psw = PS.tile([16, 4], F32, tag="A")
        for s in range(4):
            nc.tensor.matmul(psw[:, s:s + 1],
                             lhsT=ident[:BC, 16 * s:16 * s + 16],
                             rhs=idx_col[:], start=True, stop=True,
                             skip_group_check=True)
        wrap16f = SM.tile([16, 4], F32, tag="wrap16f")
        nc.vector.tensor_copy(out=wrap16f[:], in_=psw[:])
        psr = PS.tile([128, 4], F32, tag="A")
        nc.tensor.matmul(psr[:], lhsT=e16[:], rhs=wrap16f[:],
                         start=True, stop=True)
        nc.vector.tensor_copy(out=gi_dyn[:], in_=psr[:])
        nc.gpsimd.dma_gather(
            out_ap=dst_sb[:].unsqueeze(1), in_ap=table.ap(),
            idxs_ap=gi_dyn[:], num_idxs=BC, num_idxs_reg=BC, elem_size=832)

    def decode_step(i):
        # ---- gather compat from the PC tables ----
        idxg = SM.tile([BC, 1], F32, tag="idxg")
        nc.vector.tensor_add(out=idxg[:], in0=sel_f[:], in1=boffg_f[:])
        cmp_sb = SCR.tile([128, 832], F32, tag="cmp_sb")
        wrap_gather(idxg, cmp_sb, dd["dpc2"], BC * 101)
        if i == 1:
            # base1 = (W1 h[b, first]) . gK, fixed for the rest of the decode
            idxb = SM.tile([BC, 1], F32, tag="idxb")
            nc.vector.tensor_add(out=idxb[:], in0=sel_f[:], in1=boffc_f[:])
            wrap_gather(idxb, base1, dd["dpc1"], BC * N)
        # compat = gather + base1 + mask (broadcast over heads)
        nc.vector.tensor_add(out=cmp_sb[:BC, :800], in0=cmp_sb[:BC, :800],
                             in1=base1[:BC, :800])
        nc.vector.scalar_tensor_tensor(
            out=cmp_sb[:BC, :800].rearrange("b (h n) -> b h n", h=H),
            in0=cmp_sb[:BC, :800].rearrange("b (h n) -> b h n", h=H), scalar=1.0,
            in1=maskL[:].unsqueeze(1).broadcast_to([BC, H, N]),
            op0=OP.mult, op1=OP.add)
        # ---- softmax (unnormalized exp + per-head sums) ----
        nc.scalar.activation(
            out=att_sb[:].rearrange("b (h n) -> b h n", n=NPAD)[:, :, :N],
            in_=cmp_sb[:BC, :800].rearrange("b (h n) -> b h n", n=N),
            func=AF.Exp, scale=NF)
        s8 = SM.tile([BC, H], F32, tag="s8")
        nc.vector.tensor_reduce(
            out=s8[:], in_=att_sb[:].rearrange("b (h n) -> b h n", n=NPAD),
            axis=mybir.AxisListType.X, op=OP.add)
        nc.vector.reciprocal(out=s8[:], in_=s8[:])
        nc.vector.scalar_tensor_tensor(
            out=att_sb[:].rearrange("b (h n) -> b h n", n=NPAD),
            in0=att_sb[:].rearrange("b (h n) -> b h n", n=NPAD),
            scalar=1.0, in1=s8[:].unsqueeze(2).broadcast_to([BC, H, NPAD]),
            op0=OP.mult, op1=OP.mult)
        # ---- transpose att: 8 blocks [BC, 128] -> attT2 [128, (h, BC)] ----
        for hg in range(2):
            pst = PS.tile([128, 4 * BC], F32, tag="A")
            for hh in range(4):
                h = 4 * hg + hh
                nc.tensor.transpose(pst[:, hh * BC:(hh + 1) * BC],
                                    att_sb[:, h * NPAD:(h + 1) * NPAD],
                                    ident[:BC, :BC])
            # write in (quad, h, j) layout: col = 32*quad + 4*h + j, b = 4q+j
            dst = attT2[:].rearrange("p (q hh j) -> p q hh j", q=16,
                                     hh=H)[:, :, 4 * hg:4 * hg + 4, :]
            src_ap = pst[:].rearrange("p (hh q j) -> p q hh j", hh=4, q=16)
            if hg == 0:
                nc.scalar.copy(out=dst, in_=src_ap)
            else:
                nc.vector.tensor_copy(out=dst, in_=src_ap)
        # ---- glimpse: 16 quad matmuls (rhs = 4 samples' gV at once) ----
        for t in range(4):
            psg = PS.tile([128, 512], F32, tag="A")
            for qm in range(4):
                q = 4 * t + qm
                lhsT = attT2[:N, 32 * q:32 * q + 32]
                nc.tensor.matmul(
                    psg[32 * qm:32 * qm + 32, :],
                    lhsT=lhsT, rhs=gV_nat[:, 4 * q * D:(4 * q + 4) * D],
                    start=True, stop=True, tile_position=(0, 32 * qm),
                    skip_group_check=True)
            if t % 2 == 0:
                nc.scalar.copy(out=G_sb[:, t * 512:(t + 1) * 512], in_=psg[:])
            else:
                nc.vector.tensor_copy(out=G_sb[:, t * 512:(t + 1) * 512],
                                      in_=psg[:])
        # ---- transpose G (16 blocks) + diag-gather + out-proj ----
        for bgrp in range(4):
            pst = PS.tile([128, 512], F32, tag="A")
            for bb in range(4):
                blk = 4 * bgrp + bb
                nc.tensor.transpose(pst[:, bb * 128:(bb + 1) * 128],
                                    G_sb[:, blk * 128:(blk + 1) * 128],
                                    ident[:])
            if bgrp % 2 == 0:
                nc.scalar.copy(out=G_T[:, bgrp * 512:(bgrp + 1) * 512],
                               in_=pst[:])
            else:
                nc.vector.tensor_copy(out=G_T[:, bgrp * 512:(bgrp + 1) * 512],
                                      in_=pst[:])
        glT = SM.tile([128, BC], F32, tag="glT")
        nc.gpsimd.ap_gather(
            out_ap=glT[:].unsqueeze(2), in_ap=G_T[:].unsqueeze(2),
            idxs_ap=gidx_sb[:], channels=128, num_elems=16 * 128, d=1,
            num_idxs=BC)
        psp = PS.tile([D, BC], F32, tag="A")
        nc.tensor.matmul(psp[:], lhsT=w_outd[:], rhs=glT[:], start=True,
                         stop=True)
        nc.vector.tensor_copy(out=gpT[:], in_=psp[:])
        # ---- logits ----
        psl = PS.tile([N, BC], F32, tag="A")
        for b in range(BC):
            nc.tensor.matmul(psl[:, b:b + 1], lhsT=lKT[:, b * N:(b + 1) * N],
                             rhs=gpT[:, b:b + 1], start=True, stop=True,
                             skip_group_check=True)
        lT_sb = SM.tile([N, BC], F32, tag="lT_sb")
        nc.vector.tensor_copy(out=lT_sb[:], in_=psl[:])
        psl2 = PS.tile([BC, N], F32, tag="A")
        nc.tensor.transpose(psl2[:], lT_sb[:], ident[:N, :N])
        logits = SM.tile([BC, N], F32, tag="logits")
        nc.scalar.activation(out=logits[:], in_=psl2[:], func=AF.Tanh, scale=NF)
        nc.vector.scalar_tensor_tensor(
            out=logits[:], in0=logits[:], scalar=CLIP, in1=maskL[:],
            op0=OP.mult, op1=OP.add)
        # ---- argmax / lse / ll ----
        if debug and i == 0:
            nc.sync.dma_start(out=dd["ddbg_l"].ap(), in_=logits[:])
            nc.sync.dma_start(out=dd["ddbg_g"].ap(), in_=gpT[:])
        mx8 = SM.tile([BC, 8], F32, tag="mx8")
        ix8 = SM.tile([BC, 8], U32, tag="ix8")
        nc.vector.max_with_indices(mx8[:], ix8[:], logits[:])
        negmx = SM.tile([BC, 1], F32, tag="negmx")
        nc.vector.tensor_scalar_mul(out=negmx[:], in0=mx8[:, 0:1], scalar1=-1.0)
        esc = SM.tile([BC, N], F32, tag="esc")
        nc.scalar.activation(out=esc[:], in_=logits[:], func=AF.Exp,
                             bias=negmx[:], scale=1.0,
                             accum_out=sume_hist[:, i:i + 1])
        nc.vector.tensor_copy(out=sel_f[:], in_=ix8[:, 0:1])
        nc.vector.tensor_copy(out=pi_sb[:, i:i + 1], in_=ix8[:, 0:1])
        # ---- mask update ----
        onel = SM.tile([BC, N], F32, tag="onel")
        nc.vector.tensor_scalar(
            out=onel[:], in0=iota_f[:BC, :], scalar1=sel_f[:],
            scalar2=float(NEG), op0=OP.is_equal, op1=OP.mult)
        nc.vector.tensor_add(out=maskL[:], in0=maskL[:], in1=onel[:])

    for i in range(N):
        decode_step(i)

    lnh = SM.tile([BC, N], F32, tag="esc")
    nc.scalar.activation(out=lnh[:], in_=sume_hist[:], func=AF.Ln)
    nc.vector.tensor_reduce(out=ll_acc[:], in_=lnh[:],
                            axis=mybir.AxisListType.X, op=OP.add)
    nc.vector.tensor_scalar_mul(out=ll_acc[:], in0=ll_acc[:], scalar1=-1.0)
    nc.sync.dma_start(out=dll.ap().rearrange("(b o) -> b o", o=1), in_=ll_acc[:])
    pi_i32 = SM.tile([BC, N], I32, tag="pi_i32")
    nc.vector.tensor_copy(out=pi_i32[:], in_=pi_sb[:])
    nc.sync.dma_start(out=dpi.ap(), in_=pi_i32[:])


def make_e4():
    t = np.zeros((4, 128), np.float32)
    for s in range(4):
        t[s, 32 * s:32 * s + 8] = NEG
    return t


def make_e16():
    t = np.zeros((16, 128), np.float32)
    for g in range(8):
        for p in range(16):
            t[p, 16 * g + p] = 1.0
    return t


def make_hmask():
    t = np.zeros((128, 8), np.float32)
    for h in range(8):
        t[16 * h:16 * h + 16, h] = 1.0
    return t


def make_gidx():
    tbl = np.zeros((128, BC // 16), np.int16)
    for grp in range(8):
        for i in range(BC):
            p, slot = i % 16, i // 16
            t, qm, j = i // 16, (i % 16) // 4, i % 4
            tbl[16 * grp + p, slot] = 128 * (4 * t + j) + 32 * qm + 4 * grp + j
    return tbl


_CACHE = {}


def get_nc(debug=False):
    key = bool(debug)
    if key not in _CACHE:
        _CACHE[key] = build_nc(debug=debug)
    return _CACHE[key]


def host_cost(x, pi):
    d = np.take_along_axis(x, np.broadcast_to(pi[:, :, None],
                                              (pi.shape[0], N, 2)), 1)
    return (np.linalg.norm(d[:, 1:] - d[:, :-1], axis=-1).sum(1)
            + np.linalg.norm(d[:, 0] - d[:, -1], axis=-1)).astype(np.float32)


def kernel(trace=False, **inputs):
    nc = get_nc(debug=False)
    gidx = make_gidx()
    x_full = np.ascontiguousarray(np.asarray(inputs["x"], np.float32))
    in_maps = []
    for c in range(NCORE):
        m = {k: np.ascontiguousarray(np.asarray(v, dtype=np.float32))
             for k, v in inputs.items() if k != "x"}
        m["x"] = np.ascontiguousarray(x_full[c * BC:(c + 1) * BC])
        m["gidx"] = gidx
        m["e4c"] = make_e4()
        m["hmask"] = make_hmask()
        m["e16c"] = make_e16()
        in_maps.append(m)
    res = run_bass_kernel_spmd(nc, in_maps, core_ids=list(range(NCORE)),
                               trace=trace)
    if trace:
        print("exec_time_ns:", res.exec_time_ns)
        print("trace:", res.instructions_and_trace[1]
              if res.instructions_and_trace else None)
        print("profile_json:", res.profile_json)
        import json
        with open("/root/problem/trace_info.json", "w") as f:
            json.dump({"exec_time_ns": res.exec_time_ns,
                       "profile_json": res.profile_json,
                       "trace": res.instructions_and_trace[1]
                       if res.instructions_and_trace else None}, f)
    ll = np.concatenate([r["ll"] for r in res.results])
    pi = np.concatenate([r["pi"] for r in res.results]).astype(np.int32)
    cost = host_cost(x_full, pi)
    return cost, ll, pi


# revision 47
# speedup vs baseline: 1.3562x; 1.0545x over previous
"""Trainium2 Bass kernel for nn_AttentionModel (Kool-style TSP attention model).

Data-parallel over 8 NeuronCores: each core processes B/8 = 64 samples,
fp32 throughout, transposed activation layouts (features on partitions).
The TSP tour cost is reconstructed on the host from pi (pure indexing).
"""

import numpy as np

import concourse.bass as bass
import concourse.mybir as mybir
import concourse.tile as tile
from concourse import bacc
from concourse.bass_utils import run_bass_kernel_spmd
from concourse.masks import make_identity

F32 = mybir.dt.float32
I32 = mybir.dt.int32
U32 = mybir.dt.uint32
I16 = mybir.dt.int16
AF = mybir.ActivationFunctionType
OP = mybir.AluOpType

B, N, D, H, L, FF = 512, 100, 128, 8, 2, 512
DK = D // H          # 16
NCORE = 8
BC = B // NCORE      # 64 samples per core
BN = BC * N          # 6400
NPAD = 128
CLIP = 10.0
NEG = -1e9
NF = 1.0 / float(np.sqrt(D))      # decoder norm factor
SC = 1.0 / float(np.sqrt(DK))     # encoder attention scale
EPS = 1e-5

CH = [(o, min(512, BN - o)) for o in range(0, BN, 512)]


def build_nc(debug=False, use_cc=True):
    nc = bacc.Bacc("TRN2", target_bir_lowering=False, debug=False,
                   num_devices=NCORE if use_cc else 1)

    dx = nc.dram_tensor("x", [BC, N, 2], F32, kind="ExternalInput")
    dinit_W = nc.dram_tensor("init_W", [2, D], F32, kind="ExternalInput")
    dinit_b = nc.dram_tensor("init_b", [D], F32, kind="ExternalInput")
    dqkv = nc.dram_tensor("enc_qkv_W", [L, D, 3 * D], F32, kind="ExternalInput")
    doutW = nc.dram_tensor("enc_out_W", [L, D, D], F32, kind="ExternalInput")
    dbn1 = nc.dram_tensor("enc_bn1", [L, 2, D], F32, kind="ExternalInput")
    dbn2 = nc.dram_tensor("enc_bn2", [L, 2, D], F32, kind="ExternalInput")
    dff1W = nc.dram_tensor("enc_ff1_W", [L, D, FF], F32, kind="ExternalInput")
    dff1b = nc.dram_tensor("enc_ff1_b", [L, FF], F32, kind="ExternalInput")
    dff2W = nc.dram_tensor("enc_ff2_W", [L, FF, D], F32, kind="ExternalInput")
    dff2b = nc.dram_tensor("enc_ff2_b", [L, D], F32, kind="ExternalInput")
    dWph = nc.dram_tensor("W_placeholder", [2 * D], F32, kind="ExternalInput")
    dWnod = nc.dram_tensor("proj_nodes_W", [D, 3 * D], F32, kind="ExternalInput")
    dWfix = nc.dram_tensor("proj_fixed_W", [D, D], F32, kind="ExternalInput")
    dWstep = nc.dram_tensor("proj_step_W", [2 * D, D], F32, kind="ExternalInput")
    dWout = nc.dram_tensor("proj_out_W", [D, D], F32, kind="ExternalInput")
    dgidx = nc.dram_tensor("gidx", [128, BC // 16], I16, kind="ExternalInput")
    de4 = nc.dram_tensor("e4c", [4, 128], F32, kind="ExternalInput")
    dhm = nc.dram_tensor("hmask", [128, 8], F32, kind="ExternalInput")
    de16 = nc.dram_tensor("e16c", [16, 128], F32, kind="ExternalInput")

    dll = nc.dram_tensor("ll", [BC], F32, kind="ExternalOutput")
    dpi = nc.dram_tensor("pi", [BC, N], I32, kind="ExternalOutput")
    if debug:
        ddbg = nc.dram_tensor("dbg_h", [D, BN], F32, kind="ExternalOutput")
        ddbg_l = nc.dram_tensor("dbg_l", [BC, N], F32, kind="ExternalOutput")
        ddbg_g = nc.dram_tensor("dbg_g", [D, BC], F32, kind="ExternalOutput")
        ddbg_a = nc.dram_tensor("dbg_a", [128, 16 * NPAD], F32,
                                kind="ExternalOutput")
    dpc2 = nc.dram_tensor("pc2", [BC * 101, 832], F32)
    dpc1 = nc.dram_tensor("pc1", [BC * N, 832], F32)
    cc_bufs = []
    if use_cc:
        for i in range(2 * L):
            cc_bufs.append((nc.dram_tensor(f"ccin{i}", [D, 2], F32),
                            nc.dram_tensor(f"ccout{i}", [D, 2], F32)))

    with tile.TileContext(nc) as tc:
        with tc.tile_pool(name="P", bufs=1) as P, \
             tc.tile_pool(name="SCR", bufs=1) as SCR, \
             tc.tile_pool(name="SM", bufs=2) as SM, \
             tc.tile_pool(name="PS", bufs=3, space="PSUM") as PS:
            _build_model(nc, tc, P, SCR, SM, PS, locals(), debug, use_cc,
                         cc_bufs)
    nc.compile()
    return nc


def _build_model(nc, tc, P, SCR, SM, PS, dd, debug, use_cc, cc_bufs):
    dx, dinit_W, dinit_b = dd["dx"], dd["dinit_W"], dd["dinit_b"]
    dqkv, doutW, dbn1, dbn2 = dd["dqkv"], dd["doutW"], dd["dbn1"], dd["dbn2"]
    dff1W, dff1b, dff2W, dff2b = dd["dff1W"], dd["dff1b"], dd["dff2W"], dd["dff2b"]
    dWph, dWnod, dWfix, dWstep, dWout = (dd["dWph"], dd["dWnod"], dd["dWfix"],
                                         dd["dWstep"], dd["dWout"])
    dgidx, dll, dpi = dd["dgidx"], dd["dll"], dd["dpi"]

    # ---------------- weights ----------------
    ident = P.tile([128, 128], F32, tag="ident")
    make_identity(nc, ident)

    w_init = P.tile([2, D], F32, tag="w_init")
    nc.sync.dma_start(out=w_init, in_=dinit_W.ap())
    b_init = P.tile([D, 1], F32, tag="b_init")
    nc.sync.dma_start(out=b_init, in_=dinit_b.ap().rearrange("(d o) -> d o", o=1))
    w_qkv = P.tile([D, L, 3 * D], F32, tag="w_qkv")
    nc.sync.dma_start(out=w_qkv[:], in_=dqkv.ap().transpose([1, 0, 2]))
    w_out = P.tile([D, L, D], F32, tag="w_out")
    nc.sync.dma_start(out=w_out[:], in_=doutW.ap().transpose([1, 0, 2]))
    w_ff1 = P.tile([D, L, FF], F32, tag="w_ff1")
    nc.sync.dma_start(out=w_ff1[:], in_=dff1W.ap().transpose([1, 0, 2]))
    b_ff1 = P.tile([128, L, FF // 128], F32, tag="b_ff1")
    nc.sync.dma_start(
        out=b_ff1[:],
        in_=dff1b.ap().rearrange("l (c p) -> p l c", p=128))
    w_ff2 = P.tile([128, L, FF // 128, D], F32, tag="w_ff2")
    nc.sync.dma_start(
        out=w_ff2[:],
        in_=dff2W.ap().rearrange("l (c p) d -> p l c d", p=128))
    b_ff2 = P.tile([D, L], F32, tag="b_ff2")
    nc.sync.dma_start(out=b_ff2[:], in_=dff2b.ap().transpose([1, 0]))
    bn_gb1 = P.tile([D, L, 2], F32, tag="bn_gb1")  # [d, l, gamma/beta]
    nc.sync.dma_start(out=bn_gb1[:], in_=dbn1.ap().transpose([2, 0, 1]))
    bn_gb2 = P.tile([D, L, 2], F32, tag="bn_gb2")
    nc.sync.dma_start(out=bn_gb2[:], in_=dbn2.ap().transpose([2, 0, 1]))
    w_nod = P.tile([D, 3 * D], F32, tag="w_nod")
    nc.sync.dma_start(out=w_nod[:], in_=dWnod.ap())
    w_fix = P.tile([D, D], F32, tag="w_fix")
    nc.sync.dma_start(out=w_fix[:], in_=dWfix.ap())
    w_step = P.tile([128, 2, D], F32, tag="w_step")
    nc.sync.dma_start(out=w_step[:],
                      in_=dWstep.ap().rearrange("(c p) d -> p c d", p=128))
    w_outd = P.tile([D, D], F32, tag="w_outd")
    nc.sync.dma_start(out=w_outd[:], in_=dWout.ap())
    wph = P.tile([128, 2], F32, tag="wph")
    nc.sync.dma_start(out=wph[:], in_=dWph.ap().rearrange("(c p) -> p c", p=128))
    gidx_sb = P.tile([128, BC // 16], I16, tag="gidx")
    nc.sync.dma_start(out=gidx_sb[:], in_=dgidx.ap())

    # mask injector: e4[s, 32s+h] = NEG for h<8 (host-provided)
    e4 = P.tile([4, 128], F32, tag="e4")
    nc.sync.dma_start(out=e4[:], in_=dd["de4"].ap())
    hmask = P.tile([128, 8], F32, tag="hmask")
    nc.sync.dma_start(out=hmask[:], in_=dd["dhm"].ap())
    e16 = P.tile([16, 128], F32, tag="e16")
    nc.sync.dma_start(out=e16[:], in_=dd["de16"].ap())
    iota_n = P.tile([64, N], I32, tag="iota_n")
    nc.gpsimd.iota(iota_n, pattern=[[1, N]], base=0, channel_multiplier=0)
    iota_f = P.tile([64, N], F32, tag="iota_f")
    nc.vector.tensor_copy(out=iota_f[:], in_=iota_n[:])
    # row of b*100 offsets [1, 64]
    boff_i = P.tile([1, BC], I32, tag="boff_i")
    nc.gpsimd.iota(boff_i, pattern=[[N, BC]], base=0, channel_multiplier=0)
    boff_f = P.tile([1, BC], F32, tag="boff_f")
    nc.vector.tensor_copy(out=boff_f[:], in_=boff_i[:])
    boffc_i = P.tile([BC, 1], I32, tag="boffc_i")
    nc.gpsimd.iota(boffc_i, pattern=[[1, 1]], base=0, channel_multiplier=N)
    boffc_f = P.tile([BC, 1], F32, tag="boffc_f")
    nc.vector.tensor_copy(out=boffc_f[:], in_=boffc_i[:])

    eps_t = P.tile([D, 1], F32, tag="eps_t")
    nc.vector.memset(eps_t, EPS)

    hT = P.tile([D, BN], F32, tag="hT")

    # ---------------- h0 = (x @ init_W + init_b)^T ----------------
    for (o, w) in CH:
        xTs = SCR.tile([2, 512], F32, tag="xm")
        nc.sync.dma_start(
            out=xTs[:, :w],
            in_=bass.AP(tensor=dx.ap().tensor, offset=2 * o,
                        ap=[[1, 2], [2, w]]))
        ps = PS.tile([128, 512], F32, tag="A")
        nc.tensor.matmul(ps[:, :w], lhsT=w_init[:], rhs=xTs[:, :w],
                         start=True, stop=True)
        nc.scalar.activation(out=hT[:, o:o + w], in_=ps[:, :w],
                             func=AF.Identity, bias=b_init[:], scale=1.0)

    # ---------------- BatchNorm helper (in place on [D, BN]) ----------------
    def bn_pass(t, l, which):
        nsub = BN // 128  # 50 subgroups of 128 for bn_stats
        stats = SM.tile([D, nsub, 6], F32, tag="bnstats")
        tv = t[:].rearrange("d (s c) -> d s c", c=128)
        for s in range(nsub):
            nc.vector.bn_stats(out=stats[:, s, :], in_=tv[:, s, :])
        mv = SM.tile([D, 2], F32, tag="bnmv")
        nc.vector.bn_aggr(out=mv[:], in_=stats[:])
        if use_cc:
            # cross-core stats: allreduce (mean, var + mean^2), divide by 8
            pay = SM.tile([D, 2], F32, tag="ccpay")
            nc.vector.tensor_copy(out=pay[:, 0:1], in_=mv[:, 0:1])
            nc.vector.scalar_tensor_tensor(
                out=pay[:, 1:2], in0=mv[:, 0:1], scalar=mv[:, 0:1],
                in1=mv[:, 1:2], op0=OP.mult, op1=OP.add)
            cin, cout = cc_bufs[2 * l + which]
            nc.sync.dma_start(out=cin.ap(), in_=pay[:])
            nc.gpsimd.collective_compute(
                "AllReduce", OP.add, replica_groups=[list(range(NCORE))],
                ins=[cin.ap()], outs=[cout.ap()])
            nc.sync.dma_start(out=pay[:], in_=cout.ap())
            # mean = pay0/8 ; var = pay1/8 - mean^2
            nc.scalar.mul(out=mv[:, 0:1], in_=pay[:, 0:1], mul=1.0 / NCORE)
            msq = SM.tile([D, 1], F32, tag="ccmsq")
            nc.vector.tensor_mul(out=msq[:], in0=mv[:, 0:1], in1=mv[:, 0:1])
            nc.vector.tensor_scalar(
                out=mv[:, 1:2], in0=pay[:, 1:2], scalar1=1.0 / NCORE,
                scalar2=msq[:], op0=OP.mult, op1=OP.subtract)
        rstd = SM.tile([D, 1], F32, tag="bnrstd")
        nc.scalar.activation(out=rstd[:], in_=mv[:, 1:2], func=AF.Sqrt,
                             bias=eps_t[:], scale=1.0)
        nc.vector.reciprocal(out=rstd[:], in_=rstd[:])
        scale = SM.tile([D, 1], F32, tag="bnscale")
        nc.vector.tensor_mul(out=scale[:], in0=rstd[:],
                             in1=(bn_gb1 if which == 0 else bn_gb2)[:, l, 0:1])
        shift = SM.tile([D, 1], F32, tag="bnshift")
        nc.vector.tensor_mul(out=shift[:], in0=mv[:, 0:1], in1=scale[:])
        nc.vector.tensor_scalar(
            out=shift[:], in0=(bn_gb1 if which == 0 else bn_gb2)[:, l, 1:2], scalar1=shift[:],
            scalar2=None, op0=OP.subtract)
        for i, (o, w) in enumerate(CH):
            if i % 2 == 0:
                nc.scalar.activation(out=t[:, o:o + w], in_=t[:, o:o + w],
                                     func=AF.Identity, bias=shift[:],
                                     scale=scale[:])
            else:
                nc.vector.tensor_scalar(
                    out=t[:, o:o + w], in0=t[:, o:o + w], scalar1=scale[:],
                    scalar2=shift[:], op0=OP.mult, op1=OP.add)

    # ---------------- encoder ----------------
    for l in range(L):
        qT = SCR.tile([D, BN], F32, tag="qT")
        kT = SCR.tile([D, BN], F32, tag="kT")
        for blk, dst in ((0, qT), (1, kT)):
            for i, (o, w) in enumerate(CH):
                ps = PS.tile([128, 512], F32, tag="A")
                nc.tensor.matmul(ps[:, :w],
                                 lhsT=w_qkv[:, l, blk * 128:(blk + 1) * 128],
                                 rhs=hT[:, o:o + w], start=True, stop=True)
                if i % 2 == 0:
                    nc.scalar.copy(out=dst[:, o:o + w], in_=ps[:, :w])
                else:
                    nc.vector.tensor_copy(out=dst[:, o:o + w], in_=ps[:, :w])
        v_nat = SCR.tile([N, BC * D], F32, tag="v_nat")
        for b in range(BC):
            ps = PS.tile([N, 128], F32, tag="A")
            nc.tensor.matmul(ps[:], lhsT=hT[:, b * N:(b + 1) * N],
                             rhs=w_qkv[:, l, 256:384], start=True, stop=True)
            if b % 2 == 0:
                nc.scalar.copy(out=v_nat[:, b * D:(b + 1) * D], in_=ps[:])
            else:
                nc.vector.tensor_copy(out=v_nat[:, b * D:(b + 1) * D], in_=ps[:])

        # attention in groups of 4 samples (zero-embedded khat)
        GS = 2
        khat = SCR.tile([128, GS * 800], F32, tag="khat")
        nc.vector.memset(khat, 0.0)
        for bg in range(BC // GS):
            b0 = bg * GS
            for h in range(H):
                nc.vector.tensor_scalar_mul(
                    out=khat[:, :].rearrange(
                        "p (bb m) -> p bb m", bb=GS)[:, :, 100 * h:100 * h + 100],
                    in0=kT[:, b0 * N:(b0 + GS) * N].rearrange(
                        "p (bb m) -> p bb m", bb=GS),
                    scalar1=hmask[:, h:h + 1])
            att_s = SCR.tile([N, GS * 800], F32, tag="att_s")
            for bb in range(GS):
                b = b0 + bb
                ps = PS.tile([N, 800], F32, tag="A")
                for (o, w) in ((0, 512), (512, 288)):
                    nc.tensor.matmul(
                        ps[:, o:o + w], lhsT=qT[:, b * N:(b + 1) * N],
                        rhs=khat[:, bb * 800 + o: bb * 800 + o + w],
                        start=True, stop=True, skip_group_check=True)
                nc.scalar.activation(out=att_s[:, bb * 800:(bb + 1) * 800],
                                     in_=ps[:], func=AF.Exp, scale=SC)
            ssum = SM.tile([N, GS * 8], F32, tag="ssum")
            nc.vector.tensor_reduce(
                out=ssum[:],
                in_=att_s[:].rearrange("p (bb h m) -> p (bb h) m", bb=GS, h=H),
                axis=mybir.AxisListType.X, op=OP.add)
            nc.vector.reciprocal(out=ssum[:], in_=ssum[:])
            nc.vector.scalar_tensor_tensor(
                out=att_s[:].rearrange("p (bb h m) -> p (bb h) m", bb=GS, h=H),
                in0=att_s[:].rearrange("p (bb h m) -> p (bb h) m", bb=GS, h=H),
                scalar=1.0,
                in1=ssum[:].unsqueeze(2).broadcast_to([N, GS * 8, 100]),
                op0=OP.mult, op1=OP.mult)
            # transpose att blocks (batched per 4 heads); o = att @ v
            o_all = SCR.tile([N, GS * D], F32, tag="o_all")
            for bb in range(GS):
                b = b0 + bb
                atT = SCR.tile([100, 800], F32, tag="xm")
                for hg in range(2):
                    pst = PS.tile([128, 400], F32, tag="A")
                    for hh in range(4):
                        h = 4 * hg + hh
                        nc.tensor.transpose(
                            pst[:100, hh * 100:(hh + 1) * 100],
                            att_s[:, bb * 800 + h * 100: bb * 800 + (h + 1) * 100],
                            ident[:100, :100])
                    if hg == 0:
                        nc.scalar.copy(out=atT[:, :400], in_=pst[:100, :])
                    else:
                        nc.vector.tensor_copy(out=atT[:, 400:], in_=pst[:100, :])
                pso = PS.tile([N, D], F32, tag="A")
                for h in range(H):
                    nc.tensor.matmul(
                        pso[:, h * DK:(h + 1) * DK],
                        lhsT=atT[:, h * 100:(h + 1) * 100],
                        rhs=v_nat[:, b * D + h * DK: b * D + (h + 1) * DK],
                        start=True, stop=True, skip_group_check=True)
                nc.vector.tensor_copy(out=o_all[:, bb * D:(bb + 1) * D],
                                      in_=pso[:])
            # transpose o per sample and project + residual into hT
            for bb in range(GS):
                b = b0 + bb
                pst = PS.tile([128, 100], F32, tag="A")
                nc.tensor.transpose(pst[:], o_all[:, bb * D:(bb + 1) * D],
                                    ident[:100, :100])
                oTs = SM.tile([128, 100], F32, tag="oTs")
                nc.vector.tensor_copy(out=oTs[:], in_=pst[:])
                ps2 = PS.tile([128, 100], F32, tag="A")
                nc.tensor.matmul(ps2[:], lhsT=w_out[:, l, :], rhs=oTs[:],
                                 start=True, stop=True)
                nc.vector.tensor_add(out=hT[:, b * N:(b + 1) * N],
                                     in0=ps2[:], in1=hT[:, b * N:(b + 1) * N])
        bn_pass(hT, l, 0)

        # FF block (in place on hT)
        for (o, w) in CH:
            ffc = SCR.tile([128, 4, 512], F32, tag="ffc")
            for c in range(4):
                ps = PS.tile([128, 512], F32, tag="A")
                nc.tensor.matmul(ps[:, :w],
                                 lhsT=w_ff1[:, l, c * 128:(c + 1) * 128],
                                 rhs=hT[:, o:o + w], start=True, stop=True)
                nc.scalar.activation(out=ffc[:, c, :w], in_=ps[:, :w],
                                     func=AF.Relu, bias=b_ff1[:, l, c:c + 1],
                                     scale=1.0)
            ps2 = PS.tile([128, 512], F32, tag="A")
            for c in range(4):
                nc.tensor.matmul(ps2[:, :w], lhsT=w_ff2[:, l, c, :],
                                 rhs=ffc[:, c, :w],
                                 start=(c == 0), stop=(c == 3))
            nc.vector.scalar_tensor_tensor(
                out=hT[:, o:o + w], in0=ps2[:, :w], scalar=b_ff2[:, l:l + 1],
                in1=hT[:, o:o + w], op0=OP.add, op1=OP.add)
        bn_pass(hT, l, 1)

    if debug:
        nc.sync.dma_start(out=dd["ddbg"].ap(), in_=hT[:])

    # ---------------- decoder precompute ----------------
    hmean = SM.tile([D, BC], F32, tag="hmean")
    nc.vector.tensor_reduce(out=hmean[:],
                            in_=hT[:].rearrange("d (b n) -> d b n", b=BC),
                            axis=mybir.AxisListType.X, op=OP.add)
    nc.scalar.mul(out=hmean[:], in_=hmean[:], mul=1.0 / N)
    fixT = P.tile([D, BC], F32, tag="fixT")
    psf = PS.tile([D, BC], F32, tag="A")
    nc.tensor.matmul(psf[:], lhsT=w_fix[:], rhs=hmean[:], start=True, stop=True)
    nc.vector.tensor_copy(out=fixT[:], in_=psf[:])

    gKT = SCR.tile([D, BN], F32, tag="qT")
    lKT = SCR.tile([D, BN], F32, tag="kT")
    for blk, dst in ((0, gKT), (2, lKT)):
        for i, (o, w) in enumerate(CH):
            ps = PS.tile([128, 512], F32, tag="A")
            nc.tensor.matmul(ps[:, :w],
                             lhsT=w_nod[:, blk * 128:(blk + 1) * 128],
                             rhs=hT[:, o:o + w], start=True, stop=True)
            if i % 2 == 0:
                nc.scalar.copy(out=dst[:, o:o + w], in_=ps[:, :w])
            else:
                nc.vector.tensor_copy(out=dst[:, o:o + w], in_=ps[:, :w])
    gV_nat = SCR.tile([N, BC * D], F32, tag="v_nat")
    for b in range(BC):
        ps = PS.tile([N, 128], F32, tag="A")
        nc.tensor.matmul(ps[:], lhsT=hT[:, b * N:(b + 1) * N],
                         rhs=w_nod[:, 128:256], start=True, stop=True)
        if b % 2 == 0:
            nc.scalar.copy(out=gV_nat[:, b * D:(b + 1) * D], in_=ps[:])
        else:
            nc.vector.tensor_copy(out=gV_nat[:, b * D:(b + 1) * D], in_=ps[:])

    q0add = P.tile([D, 1], F32, tag="q0add")
    ps0 = PS.tile([D, 1], F32, tag="A")
    for c in range(2):
        nc.tensor.matmul(ps0[:], lhsT=w_step[:, c, :], rhs=wph[:, c:c + 1],
                         start=(c == 0), stop=(c == 1))
    nc.vector.tensor_copy(out=q0add[:], in_=ps0[:])

    # ---------------- PC tables: pc2 row b*101+p = (fix+W2 h[b,p]) . gK
    # (p=100 row: (fix + Wph@Wstep) . gK);  pc1 row b*100+p = (W1 h[b,p]) . gK
    q0T_all = P.tile([D, BC], F32, tag="q0T_all")
    nc.scalar.activation(out=q0T_all[:], in_=fixT[:], func=AF.Identity,
                         bias=q0add[:], scale=1.0)
    for b in range(BC):
        gkh = SCR.tile([128, 800], F32, tag="ffc")
        for h in range(H):
            nc.vector.tensor_scalar_mul(
                out=gkh[:, 100 * h:100 * h + 100],
                in0=gKT[:, b * N:(b + 1) * N], scalar1=hmask[:, h:h + 1])
        qs2 = SM.tile([128, 101], F32, tag="qs2")
        psq2 = PS.tile([128, 100], F32, tag="A")
        nc.tensor.matmul(psq2[:], lhsT=w_step[:, 1, :],
                         rhs=hT[:, b * N:(b + 1) * N], start=True, stop=True)
        nc.vector.tensor_scalar_add(out=qs2[:, :100], in0=psq2[:],
                                    scalar1=fixT[:, b:b + 1])
        nc.vector.tensor_copy(out=qs2[:, 100:101], in_=q0T_all[:, b:b + 1])
        psp2 = PS.tile([101, 800], F32, tag="A")
        for (o, w) in ((0, 512), (512, 288)):
            nc.tensor.matmul(psp2[:, o:o + w], lhsT=qs2[:],
                             rhs=gkh[:, o:o + w], start=True, stop=True,
                             skip_group_check=True)
        pcb2 = SCR.tile([101, 832], F32, tag="att_s")
        nc.vector.tensor_copy(out=pcb2[:, :800], in_=psp2[:])
        nc.vector.memset(pcb2[:, 800:832], 0.0)
        nc.sync.dma_start(out=dd["dpc2"].ap()[b * 101:(b + 1) * 101, :],
                          in_=pcb2[:])
        qs1 = SM.tile([128, 100], F32, tag="qs1")
        psq1 = PS.tile([128, 100], F32, tag="A")
        nc.tensor.matmul(psq1[:], lhsT=w_step[:, 0, :],
                         rhs=hT[:, b * N:(b + 1) * N], start=True, stop=True)
        nc.vector.tensor_copy(out=qs1[:], in_=psq1[:])
        psp1 = PS.tile([100, 800], F32, tag="A")
        for (o, w) in ((0, 512), (512, 288)):
            nc.tensor.matmul(psp1[:, o:o + w], lhsT=qs1[:],
                             rhs=gkh[:, o:o + w], start=True, stop=True,
                             skip_group_check=True)
        pcb1 = SCR.tile([101, 832], F32, tag="att_s")
        nc.vector.tensor_copy(out=pcb1[:100, :800], in_=psp1[:])
        nc.vector.memset(pcb1[:100, 800:832], 0.0)
        nc.sync.dma_start(out=dd["dpc1"].ap()[b * N:(b + 1) * N, :],
                          in_=pcb1[:100, :])

    # ---------------- decode state ----------------
    maskL = P.tile([BC, N], F32, tag="maskL")
    nc.vector.memset(maskL, 0.0)
    ll_acc = P.tile([BC, 1], F32, tag="ll_acc")
    nc.vector.memset(ll_acc, 0.0)
    pi_sb = P.tile([BC, N], U32, tag="pi_sb")
    att_sb = P.tile([BC, H * NPAD], F32, tag="att_d")
    nc.vector.memset(att_sb, 0.0)
    attT2 = P.tile([128, H * BC], F32, tag="attT2")
    G_sb = P.tile([128, 16 * 128], F32, tag="G_sb2")
    G_T = P.tile([128, 16 * 128], F32, tag="G_T2")
    gpT = P.tile([D, BC], F32, tag="gpT")
    base1 = P.tile([128, 832], F32, tag="base1")
    nc.vector.memset(base1, 0.0)
    sume_hist = P.tile([BC, N], F32, tag="sume_hist")
    sel_f = P.tile([BC, 1], F32, tag="sel_f")
    nc.vector.memset(sel_f, float(N))  # step 0 gathers the p=100 row of pc2
    gi_dyn = P.tile([128, 4], I16, tag="gi_dyn")
    boffg_i = P.tile([BC, 1], I32, tag="boffg_i")
    nc.gpsimd.iota(boffg_i, pattern=[[1, 1]], base=0, channel_multiplier=101)
    boffg_f = P.tile([BC, 1], F32, tag="boffg_f")
    nc.vector.tensor_copy(out=boffg_f[:], in_=boffg_i[:])

    def wrap_gather(idx_col, dst_sb, table, nrows):
        """idx_col [BC,1] f32 row-ids -> wrapped i16 -> dma_gather rows of
        `table` (DRAM [nrows, 800]) into dst_sb rows 0..BC."""
        psw = PS.tile([16, 4], F32, tag="A")
        for s in range(4):
            nc.tensor.matmul(psw[:, s:s + 1],
                             lhsT=ident[:BC, 16 * s:16 * s + 16],
                             rhs=idx_col[:], start=True, stop=True,
                             skip_group_check=True)
        wrap16f = SM.tile([16, 4], F32, tag="wrap16f")
        nc.vector.tensor_copy(out=wrap16f[:], in_=psw[:])
        psr = PS.tile([128, 4], F32, tag="A")
        nc.tensor.matmul(psr[:], lhsT=e16[:], rhs=wrap16f[:],
                         start=True, stop=True)
        nc.vector.tensor_copy(out=gi_dyn[:], in_=psr[:])
        nc.gpsimd.dma_gather(
            out_ap=dst_sb[:].unsqueeze(1), in_ap=table.ap(),
            idxs_ap=gi_dyn[:], num_idxs=BC, num_idxs_reg=BC, elem_size=832)

    def decode_step(i):
        # ---- gather compat from the PC tables ----
        idxg = SM.tile([BC, 1], F32, tag="idxg")
        nc.vector.tensor_add(out=idxg[:], in0=sel_f[:], in1=boffg_f[:])
        cmp_sb = SCR.tile([128, 832], F32, tag="cmp_sb")
        wrap_gather(idxg, cmp_sb, dd["dpc2"], BC * 101)
        if i == 1:
            # base1 = (W1 h[b, first]) . gK, fixed for the rest of the decode
            idxb = SM.tile([BC, 1], F32, tag="idxb")
            nc.vector.tensor_add(out=idxb[:], in0=sel_f[:], in1=boffc_f[:])
            wrap_gather(idxb, base1, dd["dpc1"], BC * N)
        # compat = gather + base1 + mask (broadcast over heads)
        nc.vector.tensor_add(out=cmp_sb[:BC, :800], in0=cmp_sb[:BC, :800],
                             in1=base1[:BC, :800])
        nc.vector.scalar_tensor_tensor(
            out=cmp_sb[:BC, :800].rearrange("b (h n) -> b h n", h=H),
            in0=cmp_sb[:BC, :800].rearrange("b (h n) -> b h n", h=H), scalar=1.0,
            in1=maskL[:].unsqueeze(1).broadcast_to([BC, H, N]),
            op0=OP.mult, op1=OP.add)
        # ---- softmax (unnormalized exp + per-head sums) ----
        nc.scalar.activation(
            out=att_sb[:].rearrange("b (h n) -> b h n", n=NPAD)[:, :, :N],
            in_=cmp_sb[:BC, :800].rearrange("b (h n) -> b h n", n=N),
            func=AF.Exp, scale=NF)
        s8 = SM.tile([BC, H], F32, tag="s8")
        nc.vector.tensor_reduce(
            out=s8[:], in_=att_sb[:].rearrange("b (h n) -> b h n", n=NPAD),
            axis=mybir.AxisListType.X, op=OP.add)
        nc.vector.reciprocal(out=s8[:], in_=s8[:])
        nc.vector.scalar_tensor_tensor(
            out=att_sb[:].rearrange("b (h n) -> b h n", n=NPAD),
            in0=att_sb[:].rearrange("b (h n) -> b h n", n=NPAD),
            scalar=1.0, in1=s8[:].unsqueeze(2).broadcast_to([BC, H, NPAD]),
            op0=OP.mult, op1=OP.mult)
        # ---- transpose att: 8 blocks [BC, 128] -> attT2 [128, (h, BC)] ----
        for hg in range(2):
            pst = PS.tile([128, 4 * BC], F32, tag="A")
            for hh in range(4):
                h = 4 * hg + hh
                nc.tensor.transpose(pst[:, hh * BC:(hh + 1) * BC],
                                    att_sb[:, h * NPAD:(h + 1) * NPAD],
                                    ident[:BC, :BC])
            # write in (quad, h, j) layout: col = 32*quad + 4*h + j, b = 4q+j
            dst = attT2[:].rearrange("p (q hh j) -> p q hh j", q=16,
                                     hh=H)[:, :, 4 * hg:4 * hg + 4, :]
            src_ap = pst[:].rearrange("p (hh q j) -> p q hh j", hh=4, q=16)
            if hg == 0:
                nc.scalar.copy(out=dst, in_=src_ap)
            else:
                nc.vector.tensor_copy(out=dst, in_=src_ap)
        # ---- glimpse: 16 quad matmuls (rhs = 4 samples' gV at once) ----
        for t in range(4):
            psg = PS.tile([128, 512], F32, tag="A")
            for qm in range(4):
                q = 4 * t + qm
                lhsT = attT2[:N, 32 * q:32 * q + 32]
                nc.tensor.matmul(
                    psg[32 * qm:32 * qm + 32, :],
                    lhsT=lhsT, rhs=gV_nat[:, 4 * q * D:(4 * q + 4) * D],
                    start=True, stop=True, tile_position=(0, 32 * qm),
                    skip_group_check=True)
            if t % 2 == 0:
                nc.scalar.copy(out=G_sb[:, t * 512:(t + 1) * 512], in_=psg[:])
            else:
                nc.vector.tensor_copy(out=G_sb[:, t * 512:(t + 1) * 512],
                                      in_=psg[:])
        # ---- transpose G (16 blocks) + diag-gather + out-proj ----
        for bgrp in range(4):
            pst = PS.tile([128, 512], F32, tag="A")
            for bb in range(4):
                blk = 4 * bgrp + bb
                nc.tensor.transpose(pst[:, bb * 128:(bb + 1) * 128],
                                    G_sb[:, blk * 128:(blk + 1) * 128],
                                    ident[:])
            if bgrp % 2 == 0:
                nc.scalar.copy(out=G_T[:, bgrp * 512:(bgrp + 1) * 512],
                               in_=pst[:])
            else:
                nc.vector.tensor_copy(out=G_T[:, bgrp * 512:(bgrp + 1) * 512],
                                      in_=pst[:])
        glT = SM.tile([128, BC], F32, tag="glT")
        nc.gpsimd.ap_gather(
            out_ap=glT[:].unsqueeze(2), in_ap=G_T[:].unsqueeze(2),
            idxs_ap=gidx_sb[:], channels=128, num_elems=16 * 128, d=1,
            num_idxs=BC)
        psp = PS.tile([D, BC], F32, tag="A")
        nc.tensor.matmul(psp[:], lhsT=w_outd[:], rhs=glT[:], start=True,
                         stop=True)
        nc.vector.tensor_copy(out=gpT[:], in_=psp[:])
        # ---- logits ----
        psl = PS.tile([N, BC], F32, tag="A")
        for b in range(BC):
            nc.tensor.matmul(psl[:, b:b + 1], lhsT=lKT[:, b * N:(b + 1) * N],
                             rhs=gpT[:, b:b + 1], start=True, stop=True,
                             skip_group_check=True)
        lT_sb = SM.tile([N, BC], F32, tag="lT_sb")
        nc.vector.tensor_copy(out=lT_sb[:], in_=psl[:])
        psl2 = PS.tile([BC, N], F32, tag="A")
        nc.tensor.transpose(psl2[:], lT_sb[:], ident[:N, :N])
        logits = SM.tile([BC, N], F32, tag="logits")
        nc.scalar.activation(out=logits[:], in_=psl2[:], func=AF.Tanh, scale=NF)
        nc.vector.scalar_tensor_tensor(
            out=logits[:], in0=logits[:], scalar=CLIP, in1=maskL[:],
            op0=OP.mult, op1=OP.add)
        # ---- argmax / lse / ll ----
        if debug and i == 0:
            nc.sync.dma_start(out=dd["ddbg_l"].ap(), in_=logits[:])
            nc.sync.dma_start(out=dd["ddbg_g"].ap(), in_=gpT[:])
        mx8 = SM.tile([BC, 8], F32, tag="mx8")
        ix8 = SM.tile([BC, 8], U32, tag="ix8")
        nc.vector.max_with_indices(mx8[:], ix8[:], logits[:])
        negmx = SM.tile([BC, 1], F32, tag="negmx")
        nc.vector.tensor_scalar_mul(out=negmx[:], in0=mx8[:, 0:1], scalar1=-1.0)
        esc = SM.tile([BC, N], F32, tag="esc")
        nc.scalar.activation(out=esc[:], in_=logits[:], func=AF.Exp,
                             bias=negmx[:], scale=1.0,
                             accum_out=sume_hist[:, i:i + 1])
        nc.vector.tensor_copy(out=sel_f[:], in_=ix8[:, 0:1])
        nc.vector.tensor_copy(out=pi_sb[:, i:i + 1], in_=ix8[:, 0:1])
        # ---- mask update ----
        onel = SM.tile([BC, N], F32, tag="onel")
        nc.vector.tensor_scalar(
            out=onel[:], in0=iota_f[:BC, :], scalar1=sel_f[:],
            scalar2=float(NEG), op0=OP.is_equal, op1=OP.mult)
        nc.vector.tensor_add(out=maskL[:], in0=maskL[:], in1=onel[:])

    for i in range(N):
        decode_step(i)

    lnh = SM.tile([BC, N], F32, tag="esc")
    nc.scalar.activation(out=lnh[:], in_=sume_hist[:], func=AF.Ln)
    nc.vector.tensor_reduce(out=ll_acc[:], in_=lnh[:],
                            axis=mybir.AxisListType.X, op=OP.add)
    nc.vector.tensor_scalar_mul(out=ll_acc[:], in0=ll_acc[:], scalar1=-1.0)
    nc.sync.dma_start(out=dll.ap().rearrange("(b o) -> b o", o=1), in_=ll_acc[:])
    pi_i32 = SM.tile([BC, N], I32, tag="pi_i32")
    nc.vector.tensor_copy(out=pi_i32[:], in_=pi_sb[:])
    nc.sync.dma_start(out=dpi.ap(), in_=pi_i32[:])


def make_e4():
    t = np.zeros((4, 128), np.float32)
    for s in range(4):
        t[s, 32 * s:32 * s + 8] = NEG
    return t


def make_e16():
    t = np.zeros((16, 128), np.float32)
    for g in range(8):
        for p in range(16):
            t[p, 16 * g + p] = 1.0
    return t


def make_hmask():
    t = np.zeros((128, 8), np.float32)
    for h in range(8):
        t[16 * h:16 * h + 16, h] = 1.0
    return t


def make_gidx():
    tbl = np.zeros((128, BC // 16), np.int16)
    for grp in range(8):
        for i in range(BC):
            p, slot = i % 16, i // 16
            t, qm, j = i // 16, (i % 16) // 4, i % 4
            tbl[16 * grp + p, slot] = 128 * (4 * t + j) + 32 * qm + 4 * grp + j
    return tbl


_CACHE = {}


def get_nc(debug=False):
    key = bool(debug)
    if key not in _CACHE:
        _CACHE[key] = build_nc(debug=debug)
    return _CACHE[key]


def host_cost(x, pi):
    d = np.take_along_axis(x, np.broadcast_to(pi[:, :, None],
                                              (pi.shape[0], N, 2)), 1)
    return (np.linalg.norm(d[:, 1:] - d[:, :-1], axis=-1).sum(1)
            + np.linalg.norm(d[:, 0] - d[:, -1], axis=-1)).astype(np.float32)


def kernel(trace=False, **inputs):
    nc = get_nc(debug=False)
    gidx = make_gidx()
    x_full = np.ascontiguousarray(np.asarray(inputs["x"], np.float32))
    in_maps = []
    for c in range(NCORE):
        m = {k: np.ascontiguousarray(np.asarray(v, dtype=np.float32))
             for k, v in inputs.items() if k != "x"}
        m["x"] = np.ascontiguousarray(x_full[c * BC:(c + 1) * BC])
        m["gidx"] = gidx
        m["e4c"] = make_e4()
        m["hmask"] = make_hmask()
        m["e16c"] = make_e16()
        in_maps.append(m)
    res = run_bass_kernel_spmd(nc, in_maps, core_ids=list(range(NCORE)),
                               trace=trace)
    if trace:
        print("exec_time_ns:", res.exec_time_ns)
        print("trace:", res.instructions_and_trace[1]
              if res.instructions_and_trace else None)
        print("profile_json:", res.profile_json)
        import json
        with open("/root/problem/trace_info.json", "w") as f:
            json.dump({"exec_time_ns": res.exec_time_ns,
                       "profile_json": res.profile_json,
                       "trace": res.instructions_and_trace[1]
                       if res.instructions_and_trace else None}, f)
    ll = np.concatenate([r["ll"] for r in res.results])
    pi = np.concatenate([r["pi"] for r in res.results]).astype(np.int32)
    cost = host_cost(x_full, pi)
    return cost, ll, pi
